# revision 1
# baseline (speedup 1.0000x reference)
"""Cosformer attention Bass kernel for 8 trn2 NeuronCores.

Sharding: core c handles batch c//2, sequence half c%2 (1024 positions x 1
batch = 1024 tokens). Per-head linear-attention state (kv, ksum) is
AllReduce'd (bf16) between the two cores sharing a batch.

All matmuls run in bf16 with fp32 PSUM accumulation; LN stats and epilogues
in fp32. PHM weights are kron-expanded on host; LN2 affine + final residual
are folded into the output weight (Wo'' = diag(g2) @ (Wo + I)).
"""

import sys

for _p in ('/opt/trn_rl_repo',):
    if _p not in sys.path:
        sys.path.insert(0, _p)

import importlib.util as _ilu
import os

os.environ.setdefault('NEURON_RT_RESET_CORES', '1')

# The image's antenv lacks axon_hooks (needed for trace=True); register ours.
if 'antenv.axon_hooks' not in sys.modules:
    _hp = '/opt/trn_rl_repo/antenv/axon_hooks.py'
    if os.path.exists(_hp):
        _spec = _ilu.spec_from_file_location('antenv.axon_hooks', _hp)
        _mod = _ilu.module_from_spec(_spec)
        _spec.loader.exec_module(_mod)
        sys.modules['antenv.axon_hooks'] = _mod

import numpy as np
import ml_dtypes

import concourse.bass as bass
import concourse.tile as tile
from concourse import bacc, mybir
from concourse.alu_op_type import AluOpType
from concourse.bass_utils import run_bass_kernel_spmd

BF16 = ml_dtypes.bfloat16
FP32 = mybir.dt.float32
BF = mybir.dt.bfloat16
AF = mybir.ActivationFunctionType

L, N, E, H, D = 2048, 4, 1024, 16, 64
T = 1024            # tokens per core
NT = T // 128       # 8 token tiles
NK = E // 128       # 8 contraction tiles
NJ = E // 128       # 8 output-feature tiles
NCORES = 8
EPS_LN = 1e-5
EPS_ATTN = 1e-6

_BUILD_CACHE = {}


def _build_program(flags):
    """Build the SPMD Bass program. flags: (has_g1b1, has_qb, has_kb, has_vb, has_b2o)."""
    has_g1b1, has_qb, has_kb, has_vb, has_b2o = flags

    nc = bacc.Bacc("TRN2", target_bir_lowering=False, debug=False,
                   num_devices=NCORES)

    # ---- DRAM I/O ----
    d_x_tm = nc.dram_tensor('x_tm', [T, E], FP32, kind='ExternalInput')
    d_x_fm = nc.dram_tensor('x_fm', [E, T], BF, kind='ExternalInput')
    d_wq = nc.dram_tensor('wq', [E, E], BF, kind='ExternalInput')
    d_wk = nc.dram_tensor('wk', [E, E], BF, kind='ExternalInput')
    d_wv = nc.dram_tensor('wv', [E, E], BF, kind='ExternalInput')
    d_wo = nc.dram_tensor('wo2', [E, E], BF, kind='ExternalInput')
    d_sb = nc.dram_tensor('s_bcast', [128, T], BF, kind='ExternalInput')
    d_cb = nc.dram_tensor('c_bcast', [128, T], BF, kind='ExternalInput')
    d_scol = nc.dram_tensor('s_cols', [128, NT], FP32, kind='ExternalInput')
    d_ccol = nc.dram_tensor('c_cols', [128, NT], FP32, kind='ExternalInput')
    d_g1b = nc.dram_tensor('g1_b', [128, E], FP32, kind='ExternalInput') if has_g1b1 else None
    d_b1b = nc.dram_tensor('b1_b', [128, E], FP32, kind='ExternalInput') if has_g1b1 else None
    d_qbc = nc.dram_tensor('qb_cols', [128, NJ], FP32, kind='ExternalInput') if has_qb else None
    d_kbb = nc.dram_tensor('kb_b', [128, E], FP32, kind='ExternalInput') if has_kb else None
    d_vbb = nc.dram_tensor('vb_b', [128, E], FP32, kind='ExternalInput') if has_vb else None
    d_b2ob = nc.dram_tensor('b2o_b', [128, E], FP32, kind='ExternalInput') if has_b2o else None
    d_out = nc.dram_tensor('out', [T, E], FP32, kind='ExternalOutput')

    RG = [[0, 1], [2, 3], [4, 5], [6, 7]]

    with tile.TileContext(nc) as tc:
        with (
            tc.tile_pool(name='persist', bufs=1) as pp,
            tc.tile_pool(name='wpool', bufs=2) as wp,
            tc.tile_pool(name='dram', bufs=1, space='DRAM') as dp,
        ):
            # ---- constants ----
            sbt = pp.tile([128, T], BF, tag='sbt')
            cbt = pp.tile([128, T], BF, tag='cbt')
            scol = pp.tile([128, NT], FP32, tag='scol')
            ccol = pp.tile([128, NT], FP32, tag='ccol')
            eps1 = pp.tile([128, 1], FP32, tag='eps1')
            eps2 = pp.tile([128, 1], FP32, tag='eps2')
            nc.sync.dma_start(out=scol, in_=d_scol[:])
            nc.sync.dma_start(out=ccol, in_=d_ccol[:])
            nc.vector.memset(eps1, EPS_LN)
            nc.vector.memset(eps2, EPS_ATTN)
            g1b = b1b = qbc = kbb = vbb = b2ob = None
            if has_g1b1:
                g1b = pp.tile([128, E], FP32, tag='g1b')
                b1b = pp.tile([128, E], FP32, tag='b1b')
                nc.gpsimd.dma_start(out=g1b, in_=d_g1b[:])
                nc.gpsimd.dma_start(out=b1b, in_=d_b1b[:])
            if has_qb:
                qbc = pp.tile([128, NJ], FP32, tag='qbc')
                nc.gpsimd.dma_start(out=qbc, in_=d_qbc[:])
            if has_kb:
                kbb = pp.tile([128, E], FP32, tag='kbb')
                nc.gpsimd.dma_start(out=kbb, in_=d_kbb[:])
            if has_vb:
                vbb = pp.tile([128, E], FP32, tag='vbb')
                nc.gpsimd.dma_start(out=vbb, in_=d_vbb[:])
            if has_b2o:
                b2ob = pp.tile([128, E], FP32, tag='b2ob')
                nc.gpsimd.dma_start(out=b2ob, in_=d_b2ob[:])

            # persistent activation tiles
            qnT = pp.tile([128, NJ, T], BF, tag='qnT')     # qn feature-major
            kvb = pp.tile([128, H * 65], BF, tag='kvb')    # reduced kv (bf16)
            qq = pp.tile([128, H, T], BF, tag='qq')        # q_ per head, fm

            # DRAM scratch
            qn_dram = dp.tile([T, E], BF)
            xh_dram = dp.tile([T, E], BF)
            kv_cc_in = dp.tile([128, H * 65], BF)
            kv_cc_out = dp.tile([128, H * 65], BF)

            with (
                tc.tile_pool(name='xfmp', bufs=1) as xfmp,
                tc.tile_pool(name='bc', bufs=1) as bcp,
                tc.tile_pool(name='ln1', bufs=3) as ap,
            ):
                # feature-major x (sync) + Wk (scalar): per-k-tile chunks so
                # the first accumulation k-step starts as soon as chunk 0 lands
                xfm = xfmp.tile([128, NK, T], BF, tag='xfm')
                xfm_src = d_x_fm[:].rearrange('(k p) t -> p k t', p=128)
                wk_t = wp.tile([128, NK, E], BF, tag='W')
                wk_src = d_wk[:].rearrange('(k p) e -> p k e', p=128)
                for k in range(NK):
                    nc.sync.dma_start(out=xfm[:, k, :], in_=xfm_src[:, k, :])
                    nc.scalar.dma_start(out=wk_t[:, k, :], in_=wk_src[:, k, :])
                nc.sync.dma_start(out=sbt, in_=d_sb[:])
                nc.sync.dma_start(out=cbt, in_=d_cb[:])
                wv_t = wp.tile([128, NK, E], BF, tag='W')
                wv_src = d_wv[:].rearrange('(k p) e -> p k e', p=128)
                for k in range(NK):
                    nc.gpsimd.dma_start(out=wv_t[:, k, :], in_=wv_src[:, k, :])

                # [ks | kc] interleaved per head: kv needs single (128,128) lhsT
                ksc = bcp.tile([128, NT, H, 128], BF, tag='ksc')
                v_aug = bcp.tile([128, NT, H, 65], BF, tag='vaug')
                kvp = bcp.tile([128, H * 65], BF, tag='kvp')
                nc.vector.memset(v_aug[:, :, :, 64:65], 1.0)

                with tc.tile_pool(name='psB', bufs=8, space='PSUM') as psb:
                    def phm_tok_major(w_t, epilogue):
                        # k-outer in 2 half-passes of 8 PSUM banks
                        for half in range(2):
                            ptiles = {}
                            for i in range(4 * half, 4 * half + 4):
                                for ch in range(2):
                                    pt = psb.tile([128, 512], FP32, tag='psB',
                                                  name=f'pb_{i}_{ch}')
                                    ptiles[i, ch] = pt
                            for k in range(NK):
                                for i in range(4 * half, 4 * half + 4):
                                    for ch in range(2):
                                        csl = slice(ch * 512, (ch + 1) * 512)
                                        nc.tensor.matmul(
                                            ptiles[i, ch],
                                            lhsT=xfm[:, k, i * 128:(i + 1) * 128],
                                            rhs=w_t[:, k, csl],
                                            start=(k == 0), stop=(k == NK - 1))
                            for i in range(4 * half, 4 * half + 4):
                                for ch in range(2):
                                    epilogue(i, ch, ptiles[i, ch])

                    def k_epilogue(i, ch, pk):
                        if has_kb:
                            csl = slice(ch * 512, (ch + 1) * 512)
                            nc.vector.tensor_tensor(out=pk, in0=pk, in1=kbb[:, csl],
                                                    op=AluOpType.add)
                        pkv = pk[:].rearrange('p (h d) -> p h d', d=64)
                        nc.vector.tensor_scalar(
                            out=ksc[:, i, ch * 8:(ch + 1) * 8, 0:64], in0=pkv,
                            scalar1=0.0, scalar2=scol[:, i:i + 1],
                            op0=AluOpType.max, op1=AluOpType.mult)
                        nc.vector.tensor_scalar(
                            out=ksc[:, i, ch * 8:(ch + 1) * 8, 64:128], in0=pkv,
                            scalar1=0.0, scalar2=ccol[:, i:i + 1],
                            op0=AluOpType.max, op1=AluOpType.mult)

                    def v_epilogue(i, ch, pv):
                        if has_vb:
                            csl = slice(ch * 512, (ch + 1) * 512)
                            nc.vector.tensor_tensor(out=pv, in0=pv, in1=vbb[:, csl],
                                                    op=AluOpType.add)
                        nc.vector.tensor_copy(
                            out=v_aug[:, i, ch * 8:(ch + 1) * 8, 0:64],
                            in_=pv[:].rearrange('p (h d) -> p h d', d=64))

                    # ============ Phase B1: k matmuls ============
                    phm_tok_major(wk_t, k_epilogue)

                    # ============ Phase A: LN1 -> qn (x_tm on scalar q) =====
                    for i in range(NT):
                        xt = ap.tile([128, E], FP32, tag='xt')
                        nc.scalar.dma_start(out=xt, in_=d_x_tm[i * 128:(i + 1) * 128, :])
                        st = ap.tile([128, 2, 6], FP32, tag='st')
                        xg = xt[:].rearrange('p (g d) -> p g d', g=2)
                        nc.vector.bn_stats(out=st[:, 0, :], in_=xg[:, 0, :])
                        nc.vector.bn_stats(out=st[:, 1, :], in_=xg[:, 1, :])
                        mv = ap.tile([128, 2], FP32, tag='mv')
                        nc.vector.bn_aggr(out=mv, in_=st)
                        rstd = ap.tile([128, 1], FP32, tag='rstd')
                        nc.scalar.activation(out=rstd, in_=mv[:, 1:2], func=AF.Sqrt,
                                             bias=eps1, scale=1.0)
                        nc.vector.reciprocal(out=rstd, in_=rstd)
                        qnt = ap.tile([128, E], BF, tag='qnt')
                        if has_g1b1:
                            tmp = ap.tile([128, E], FP32, tag='qtmp')
                            nc.vector.tensor_scalar(out=tmp, in0=xt, scalar1=mv[:, 0:1],
                                                    scalar2=rstd, op0=AluOpType.subtract,
                                                    op1=AluOpType.mult)
                            nc.vector.tensor_mul(tmp, tmp, g1b)
                            nc.vector.tensor_tensor(out=qnt, in0=tmp, in1=b1b,
                                                    op=AluOpType.add)
                        else:
                            nc.vector.tensor_scalar(out=qnt, in0=xt, scalar1=mv[:, 0:1],
                                                    scalar2=rstd, op0=AluOpType.subtract,
                                                    op1=AluOpType.mult)
                        nc.gpsimd.dma_start(out=qn_dram[i * 128:(i + 1) * 128, :], in_=qnt)

                    # qn transposes: sync queue (all xbar ops live on sync)
                    for j in range(NJ):
                        nc.sync.dma_start(out=qnT[:, j, :],
                                          in_=qn_dram[:, j * 128:(j + 1) * 128],
                                          transpose=True)

                    # ============ Phase B2: v matmuls ============
                    phm_tok_major(wv_t, v_epilogue)

                # Wq load (Wk slot frees after B1); scalar queue
                wq_t = wp.tile([128, NK, E], BF, tag='W')
                wq_src = d_wq[:].rearrange('(k p) e -> p k e', p=128)
                for k in range(NK):
                    nc.scalar.dma_start(out=wq_t[:, k, :], in_=wq_src[:, k, :])

                # ============ Phase C: per-head kv partials + AllReduce =====
                with tc.tile_pool(name='psC', bufs=8, space='PSUM') as psc:
                    for h in range(H):
                        pkv = psc.tile([128, 65], FP32, tag='psC')
                        for i in range(NT):
                            nc.tensor.matmul(pkv, lhsT=ksc[:, i, h, :],
                                             rhs=v_aug[:, i, h, :],
                                             start=(i == 0), stop=(i == NT - 1))
                        nc.vector.tensor_copy(out=kvp[:, h * 65:(h + 1) * 65], in_=pkv)

                nc.gpsimd.dma_start(out=kv_cc_in[:], in_=kvp)
                nc.gpsimd.collective_compute(
                    'AllReduce', AluOpType.add,
                    ins=[kv_cc_in.opt()], outs=[kv_cc_out.opt()],
                    replica_groups=RG)
                nc.gpsimd.dma_start(out=kvb, in_=kv_cc_out[:])

            # Wo2 load (Wv slot frees after B2); gpsimd queue
            wo_t = wp.tile([128, NK, E], BF, tag='W')
            wo_src = d_wo[:].rearrange('(k p) e -> p k e', p=128)
            for k in range(NK):
                nc.gpsimd.dma_start(out=wo_t[:, k, :], in_=wo_src[:, k, :])

            # ======== Phases D, E, F, G interleaved (keep the PE warm) ======
            with (
                tc.tile_pool(name='gx', bufs=1) as gxp,
                tc.tile_pool(name='psDsb', bufs=3) as psd_sb,
                tc.tile_pool(name='ef', bufs=3) as efp,
            ):
                xhT = gxp.tile([128, NJ, T], BF, tag='xhT')

                def emit_q(ch, psd):
                    csl = slice(ch * 512, (ch + 1) * 512)
                    for j in range(NJ):
                        pq = psd.tile([128, 512], FP32, tag='psD', name=f'pq_{ch}_{j}')
                        for k in range(NK):
                            nc.tensor.matmul(pq, lhsT=wq_t[:, k, j * 128:(j + 1) * 128],
                                             rhs=qnT[:, k, csl],
                                             start=(k == 0), stop=(k == NK - 1))
                        qrel = psd_sb.tile([128, 512], BF, tag='qrel')
                        if has_qb:
                            nc.scalar.activation(out=qrel, in_=pq, func=AF.Relu,
                                                 bias=qbc[:, j:j + 1])
                        else:
                            nc.scalar.activation(out=qrel, in_=pq, func=AF.Relu)
                        for hh in range(2):
                            h = 2 * j + hh
                            rs = slice(hh * 64, (hh + 1) * 64)
                            nc.vector.tensor_tensor(
                                out=qq[0:64, h, csl], in0=qrel[rs, :],
                                in1=sbt[rs, csl], op=AluOpType.mult)
                            nc.vector.tensor_tensor(
                                out=qq[64:128, h, csl], in0=qrel[rs, :],
                                in1=cbt[rs, csl], op=AluOpType.mult)

                def emit_attn_ln2(i):
                    rsl = slice(i * 128, (i + 1) * 128)
                    qnr = efp.tile([128, E], BF, tag='qnr')
                    nc.gpsimd.dma_start(out=qnr, in_=qn_dram[rsl, :])
                    yt = efp.tile([128, H, 64], BF, tag='yt')
                    dcol = efp.tile([128, H], FP32, tag='dcol')
                    z16 = efp.tile([128, H], FP32, tag='z16')
                    pas = []
                    for g in range(4):
                        pa = pse.tile([128, 4 * 65], FP32, tag='psE', name=f'pa_{i}_{g}')
                        pas.append(pa)
                        for hh in range(4):
                            h = 4 * g + hh
                            nc.tensor.matmul(pa[:, hh * 65:(hh + 1) * 65],
                                             lhsT=qq[:, h, rsl],
                                             rhs=kvb[:, h * 65:(h + 1) * 65],
                                             start=True, stop=True)
                        pav = pa[:].rearrange('p (h c) -> p h c', c=65)
                        nc.vector.tensor_copy(out=dcol[:, g * 4:(g + 1) * 4],
                                              in_=pav[:, :, 64])
                    # z = 1/max(denom, eps), batched over all heads
                    nc.vector.tensor_scalar(out=z16, in0=dcol, scalar1=EPS_ATTN,
                                            scalar2=None, op0=AluOpType.max)
                    nc.vector.reciprocal(out=z16, in_=z16)
                    for g in range(4):
                        pav = pas[g][:].rearrange('p (h c) -> p h c', c=65)
                        zb = z16[:, g * 4:(g + 1) * 4].broadcast_to((128, 4, 64))
                        nc.vector.tensor_tensor(out=yt[:, g * 4:(g + 1) * 4, :],
                                                in0=pav[:, :, 0:64], in1=zb,
                                                op=AluOpType.mult)
                    ytf = yt[:].rearrange('p h d -> p (h d)')
                    nc.vector.tensor_tensor(out=ytf, in0=ytf, in1=qnr,
                                            op=AluOpType.add)
                    # LN2 (stats on bf16 y)
                    st2 = efp.tile([128, 2, 6], FP32, tag='st2')
                    yg = yt[:].rearrange('p (g x) d -> p g (x d)', g=2)
                    nc.vector.bn_stats(out=st2[:, 0, :], in_=yg[:, 0, :])
                    nc.vector.bn_stats(out=st2[:, 1, :], in_=yg[:, 1, :])
                    mv2 = efp.tile([128, 2], FP32, tag='mv2')
                    nc.vector.bn_aggr(out=mv2, in_=st2)
                    rstd2 = efp.tile([128, 1], FP32, tag='rstd2')
                    nc.scalar.activation(out=rstd2, in_=mv2[:, 1:2], func=AF.Sqrt,
                                         bias=eps1, scale=1.0)
                    nc.vector.reciprocal(out=rstd2, in_=rstd2)
                    xh = efp.tile([128, E], BF, tag='xh')
                    nc.vector.tensor_scalar(out=xh, in0=ytf, scalar1=mv2[:, 0:1],
                                            scalar2=rstd2, op0=AluOpType.subtract,
                                            op1=AluOpType.mult)
                    nc.gpsimd.dma_start(out=xh_dram[rsl, :], in_=xh)

                def emit_xh_transpose(tsl):
                    for j in range(NJ):
                        nc.sync.dma_start(out=xhT[:, j, tsl],
                                          in_=xh_dram[tsl, j * 128:(j + 1) * 128],
                                          transpose=True)

                def emit_o(i, gop, psg):
                    for ch in range(2):
                        csl = slice(ch * 512, (ch + 1) * 512)
                        po = psg.tile([128, 512], FP32, tag='psG', name=f'po_{i}_{ch}')
                        for k in range(NK):
                            nc.tensor.matmul(po, lhsT=xhT[:, k, i * 128:(i + 1) * 128],
                                             rhs=wo_t[:, k, csl],
                                             start=(k == 0), stop=(k == NK - 1))
                        ot = gop.tile([128, 512], FP32, tag='ot')
                        if has_b2o:
                            nc.vector.tensor_tensor(out=ot, in0=po,
                                                    in1=b2ob[:, csl], op=AluOpType.add)
                        else:
                            nc.vector.tensor_copy(out=ot, in_=po)
                        nc.scalar.dma_start(out=d_out[i * 128:(i + 1) * 128, csl],
                                            in_=ot)

                with tc.tile_pool(name='psD', bufs=4, space='PSUM') as psd:
                    emit_q(0, psd)
                    emit_q(1, psd)
                with (
                    tc.tile_pool(name='psE', bufs=4, space='PSUM') as pse,
                    tc.tile_pool(name='go', bufs=4) as gop,
                    tc.tile_pool(name='psG', bufs=4, space='PSUM') as psg,
                ):
                    emit_attn_ln2(0)
                    emit_attn_ln2(1)
                    emit_xh_transpose(slice(0, 256))
                    emit_attn_ln2(2)
                    emit_o(0, gop, psg)
                    emit_attn_ln2(3)
                    emit_o(1, gop, psg)
                    emit_xh_transpose(slice(256, 512))
                    emit_attn_ln2(4)
                    emit_o(2, gop, psg)
                    emit_attn_ln2(5)
                    emit_o(3, gop, psg)
                    emit_xh_transpose(slice(512, 768))
                    emit_attn_ln2(6)
                    emit_o(4, gop, psg)
                    emit_xh_transpose(slice(768, 896))
                    emit_attn_ln2(7)
                    emit_o(5, gop, psg)
                    emit_xh_transpose(slice(896, 1024))
                    emit_o(6, gop, psg)
                    emit_o(7, gop, psg)

    nc.compile()
    return nc


def _get_program(flags):
    if flags not in _BUILD_CACHE:
        _BUILD_CACHE[flags] = _build_program(flags)
    return _BUILD_CACHE[flags]


def _phm_weight(A, S):
    f = A.shape[0]
    din, dout = f * S.shape[1], f * S.shape[2]
    W = np.einsum('nij,nkl->ikjl', np.asarray(A, np.float32), np.asarray(S, np.float32))
    return np.ascontiguousarray(W.reshape(din, dout))


def kernel(**inputs):
    query = np.asarray(inputs['query'], np.float32)
    g1 = np.asarray(inputs['g1'], np.float32)
    b1 = np.asarray(inputs['b1'], np.float32)
    g2 = np.asarray(inputs['g2'], np.float32)
    b2 = np.asarray(inputs['b2'], np.float32)
    qb = np.asarray(inputs['qb'], np.float32)
    kb = np.asarray(inputs['kb'], np.float32)
    vb = np.asarray(inputs['vb'], np.float32)
    ob = np.asarray(inputs['ob'], np.float32)

    Wq = _phm_weight(inputs['qA'], inputs['qS'])
    Wk = _phm_weight(inputs['kA'], inputs['kS'])
    Wv = _phm_weight(inputs['vA'], inputs['vS'])
    Wo = _phm_weight(inputs['oA'], inputs['oS'])
    WoI = Wo + np.eye(E, dtype=np.float32)
    Wo2 = g2[:, None] * WoI
    B2O = b2 @ WoI + ob

    has_g1b1 = not (np.all(g1 == 1.0) and np.all(b1 == 0.0))
    has_qb = bool(np.any(qb != 0.0))
    has_kb = bool(np.any(kb != 0.0))
    has_vb = bool(np.any(vb != 0.0))
    has_b2o = bool(np.any(B2O != 0.0))
    flags = (has_g1b1, has_qb, has_kb, has_vb, has_b2o)

    nc = _get_program(flags)

    s_full = np.sin((np.pi / 2) * np.arange(1, L + 1, dtype=np.float32) / L)
    c_full = np.cos((np.pi / 2) * np.arange(1, L + 1, dtype=np.float32) / L)

    wq_b = Wq.astype(BF16)
    wk_b = Wk.astype(BF16)
    wv_b = Wv.astype(BF16)
    wo_b = Wo2.astype(BF16)

    in_maps = []
    for core in range(NCORES):
        b = core // 2
        l0 = (core % 2) * T
        x = np.ascontiguousarray(query[l0:l0 + T, b, :])
        s = s_full[l0:l0 + T]
        c = c_full[l0:l0 + T]
        im = {
            'x_tm': x,
            'x_fm': np.ascontiguousarray(x.T).astype(BF16),
            'wq': wq_b, 'wk': wk_b, 'wv': wv_b, 'wo2': wo_b,
            's_bcast': np.ascontiguousarray(np.broadcast_to(s, (128, T))).astype(BF16),
            'c_bcast': np.ascontiguousarray(np.broadcast_to(c, (128, T))).astype(BF16),
            's_cols': np.ascontiguousarray(s.reshape(NT, 128).T),
            'c_cols': np.ascontiguousarray(c.reshape(NT, 128).T),
        }
        if has_g1b1:
            im['g1_b'] = np.ascontiguousarray(np.broadcast_to(g1, (128, E)))
            im['b1_b'] = np.ascontiguousarray(np.broadcast_to(b1, (128, E)))
        if has_qb:
            im['qb_cols'] = np.ascontiguousarray(qb.reshape(NJ, 128).T)
        if has_kb:
            im['kb_b'] = np.ascontiguousarray(np.broadcast_to(kb, (128, E)))
        if has_vb:
            im['vb_b'] = np.ascontiguousarray(np.broadcast_to(vb, (128, E)))
        if has_b2o:
            im['b2o_b'] = np.ascontiguousarray(np.broadcast_to(B2O, (128, E)))
        in_maps.append(im)

    trace = bool(os.environ.get('KERNEL_TRACE'))
    res = run_bass_kernel_spmd(nc, in_maps, list(range(NCORES)), trace=trace)
    kernel._last_exec_ns = res.exec_time_ns

    out = np.empty((L, N, E), np.float32)
    for core in range(NCORES):
        b = core // 2
        l0 = (core % 2) * T
        out[l0:l0 + T, b, :] = res.results[core]['out']
    return out


kernel._last_exec_ns = None



# revision 10
# speedup vs baseline: 1.0662x; 1.0662x over previous
"""Cosformer attention Bass kernel for 8 trn2 NeuronCores.

Sharding: core c handles batch c//2, sequence half c%2 (1024 tokens).
Per-head linear-attention state (kv, ksum) is AllReduce'd (bf16) between
the two cores sharing a batch.

v2 design (vs baseline):
- LN1 is folded into the q GEMM: q runs W-stationary on the feature-major
  x (already in SBUF) plus a rank-1 cq (x) (-mu) correction matmul into the
  same PSUM accumulation; rstd is folded into the sin/cos multiplier tiles
  (valid because rstd, s, c > 0 commute with ReLU). No qn transposes.
- Phase order on the PE: v GEMM, k GEMM with the per-head kv matmuls
  interleaved per tile-pair (AllReduce triggers right after the k GEMM),
  q GEMM (covers the AllReduce), then attn matmuls interleaved with the
  output GEMM.
- Engine balance: LN1 emitted first on vector (overlaps the v GEMM);
  k-epilogue split scalar/vector; v-epilogue on gpsimd; attn epilogue
  split gpsimd/vector/scalar; xh transposes split across both HWDGE
  queues (sync+scalar); output DMAs on sync.
"""

import sys

for _p in ('/opt/trn_rl_repo',):
    if _p not in sys.path:
        sys.path.insert(0, _p)

import importlib.util as _ilu
import os

os.environ.setdefault('NEURON_RT_RESET_CORES', '1')

# The image's antenv may lack axon_hooks (needed for trace=True); register
# a stub module so `from antenv.axon_hooks import ...` works.
if 'antenv.axon_hooks' not in sys.modules:
    try:
        import antenv.axon_hooks  # noqa: F401
    except ImportError:
        import types as _types

        _mod = _types.ModuleType('antenv.axon_hooks')
        _mod._hook = None

        def _set_hook(h):
            _mod._hook = h

        def _get_hook():
            return _mod._hook

        _mod.set_axon_ntff_profile_hook = _set_hook
        _mod.get_axon_ntff_profile_hook = _get_hook
        sys.modules['antenv.axon_hooks'] = _mod

import numpy as np
import ml_dtypes

import concourse.bass as bass
import concourse.tile as tile
from concourse import bacc, mybir
from concourse.alu_op_type import AluOpType
from concourse.bass_utils import run_bass_kernel_spmd

BF16 = ml_dtypes.bfloat16
FP32 = mybir.dt.float32
BF = mybir.dt.bfloat16
AF = mybir.ActivationFunctionType

L, N, E, H, D = 2048, 4, 1024, 16, 64
T = 1024            # tokens per core
NT = T // 128       # 8 token tiles
NK = E // 128       # 8 contraction tiles
NJ = E // 128       # 8 output-feature tiles
NCORES = 8
EPS_LN = 1e-5
EPS_ATTN = 1e-6

_BUILD_CACHE = {}


def _build_program(flags):
    """Build the SPMD Bass program. flags: (has_g1b1, has_qb, has_kb, has_vb, has_b2o)."""
    has_g1b1, has_qb, has_kb, has_vb, has_b2o = flags

    nc = bacc.Bacc("TRN2", target_bir_lowering=False, debug=False,
                   num_devices=NCORES)

    # ---- DRAM I/O ----
    d_x_tm = nc.dram_tensor('x_tm', [T, E], FP32, kind='ExternalInput')
    d_x_fm = nc.dram_tensor('x_fm', [E, T], BF, kind='ExternalInput')
    d_wq = nc.dram_tensor('wq', [E, E], BF, kind='ExternalInput')
    d_wk = nc.dram_tensor('wk', [E, E], BF, kind='ExternalInput')
    d_wv = nc.dram_tensor('wv', [E, E], BF, kind='ExternalInput')
    d_wo = nc.dram_tensor('wo2', [E, E], BF, kind='ExternalInput')
    d_sb = nc.dram_tensor('s_bcast', [128, T], BF, kind='ExternalInput')
    d_cb = nc.dram_tensor('c_bcast', [128, T], BF, kind='ExternalInput')
    d_scol = nc.dram_tensor('s_cols', [128, NT], FP32, kind='ExternalInput')
    d_ccol = nc.dram_tensor('c_cols', [128, NT], FP32, kind='ExternalInput')
    d_cq = nc.dram_tensor('cq_row', [1, E], BF, kind='ExternalInput')
    d_g1b = nc.dram_tensor('g1_b', [128, E], FP32, kind='ExternalInput') if has_g1b1 else None
    d_b1b = nc.dram_tensor('b1_b', [128, E], FP32, kind='ExternalInput') if has_g1b1 else None
    d_qbc = nc.dram_tensor('qb_cols', [128, NJ], FP32, kind='ExternalInput') if has_qb else None
    d_kbb = nc.dram_tensor('kb_b', [128, E], FP32, kind='ExternalInput') if has_kb else None
    d_vbb = nc.dram_tensor('vb_b', [128, E], FP32, kind='ExternalInput') if has_vb else None
    d_b2ob = nc.dram_tensor('b2o_b', [128, E], FP32, kind='ExternalInput') if has_b2o else None
    d_out = nc.dram_tensor('out', [T, E], FP32, kind='ExternalOutput')

    RG = [[0, 1], [2, 3], [4, 5], [6, 7]]

    with tile.TileContext(nc) as tc:
        with (
            tc.tile_pool(name='persist', bufs=1) as pp,
            tc.tile_pool(name='wpool', bufs=2) as wp,
            tc.tile_pool(name='dram', bufs=1, space='DRAM') as dp,
        ):
            # ---- priority DMAs: first GEMM inputs ----
            xfm = pp.tile([128, NK, T], BF, tag='xfm')
            xfm_src = d_x_fm[:].rearrange('(k p) t -> p k t', p=128)
            for k in range(NK):
                nc.sync.dma_start(out=xfm[:, k, :], in_=xfm_src[:, k, :])
            wv_t = wp.tile([128, NK, E], BF, tag='W', name='wv')
            wv_src = d_wv[:].rearrange('(k p) e -> p k e', p=128)
            for k in range(NK):
                nc.gpsimd.dma_start(out=wv_t[:, k, :], in_=wv_src[:, k, :])

            # ---- constants (sync queue, after xfm) ----
            scol = pp.tile([128, NT], FP32, tag='scol')
            ccol = pp.tile([128, NT], FP32, tag='ccol')
            sbt = pp.tile([128, T], BF, tag='sbt')
            cbt = pp.tile([128, T], BF, tag='cbt')
            cq_sb = pp.tile([1, E], BF, tag='cq')
            eps1 = pp.tile([128, 1], FP32, tag='eps1')
            nc.sync.dma_start(out=scol, in_=d_scol[:])
            nc.sync.dma_start(out=ccol, in_=d_ccol[:])
            nc.sync.dma_start(out=sbt, in_=d_sb[:])
            nc.sync.dma_start(out=cbt, in_=d_cb[:])
            nc.sync.dma_start(out=cq_sb, in_=d_cq[:])
            nc.vector.memset(eps1, EPS_LN)

            # wk on sync after consts (needed when the k GEMM starts)
            wk_t = wp.tile([128, NK, E], BF, tag='W', name='wk')
            wk_src = d_wk[:].rearrange('(k p) e -> p k e', p=128)
            for k in range(NK):
                nc.sync.dma_start(out=wk_t[:, k, :], in_=wk_src[:, k, :])

            g1b = b1b = qbc = kbb = vbb = b2ob = None
            if has_g1b1:
                g1b = pp.tile([128, E], FP32, tag='g1b')
                b1b = pp.tile([128, E], FP32, tag='b1b')
                nc.gpsimd.dma_start(out=g1b, in_=d_g1b[:])
                nc.gpsimd.dma_start(out=b1b, in_=d_b1b[:])
            if has_qb:
                qbc = pp.tile([128, NJ], FP32, tag='qbc')
                nc.gpsimd.dma_start(out=qbc, in_=d_qbc[:])
            if has_kb:
                kbb = pp.tile([128, E], FP32, tag='kbb')
                nc.gpsimd.dma_start(out=kbb, in_=d_kbb[:])
            if has_vb:
                vbb = pp.tile([128, E], FP32, tag='vbb')
                nc.gpsimd.dma_start(out=vbb, in_=d_vbb[:])
            if has_b2o:
                b2ob = pp.tile([128, E], FP32, tag='b2ob')
                nc.gpsimd.dma_start(out=b2ob, in_=d_b2ob[:])

            # ---- persistent activation tiles ----
            ksc = pp.tile([128, NT, H, 128], BF, tag='ksc')    # [ks|kc] per head
            v_aug = pp.tile([128, NT, H, 65], BF, tag='vaug')  # v + ones col
            qn_sb = pp.tile([128, NT, E], BF, tag='qn')        # LN1(x) for residual
            qq = pp.tile([128, H, T], BF, tag='qq')            # q_ per head, fm
            kvb = pp.tile([128, H * 65], BF, tag='kvb')        # reduced kv
            xhT = pp.tile([128, NK, T], BF, tag='xhT')         # xh feature-major
            nmu_c = pp.tile([128, 128], BF, tag='nmu_c')       # [-mu | pad | rstd | pad] cols
            nmu_row = pp.tile([1, T], BF, tag='nmu_row')       # -mu, token-major row
            rstd_row = pp.tile([1, T], BF, tag='rstd_row')
            rstd_b = pp.tile([128, T], BF, tag='rstd_b')       # rstd bcast over parts
            srt = pp.tile([128, T], BF, tag='srt')             # s*rstd (or s)
            crt = pp.tile([128, T], BF, tag='crt')             # c*rstd (or c)

            # ---- DRAM scratch ----
            xh_dram = dp.tile([T, E], BF)
            nm_dram = dp.tile([2 * NT, 128], BF)     # transposed [-mu|rstd]
            kv_cc_in = dp.tile([128, H * 65], BF)
            kv_cc_out = dp.tile([128, H * 65], BF)

            nc.gpsimd.memset(v_aug[:, :, :, 64:65], 1.0)
            nc.gpsimd.memset(nmu_c, 0.0)

            # ============ Phase A: LN1 stats + qn (vector, overlaps B1) ====
            with tc.tile_pool(name='ln1', bufs=4) as ap:
                for i in range(NT):
                    xt = ap.tile([128, E], FP32, tag='xt')
                    nc.scalar.dma_start(out=xt, in_=d_x_tm[i * 128:(i + 1) * 128, :])
                    st = ap.tile([128, 2, 6], FP32, tag='st')
                    xg = xt[:].rearrange('p (g d) -> p g d', g=2)
                    nc.vector.bn_stats(out=st[:, 0, :], in_=xg[:, 0, :])
                    nc.vector.bn_stats(out=st[:, 1, :], in_=xg[:, 1, :])
                    mv = ap.tile([128, 2], FP32, tag='mv')
                    nc.vector.bn_aggr(out=mv, in_=st)
                    # -mu into cols tile
                    nc.vector.tensor_scalar(out=nmu_c[:, i:i + 1], in0=mv[:, 0:1],
                                            scalar1=-1.0, scalar2=None,
                                            op0=AluOpType.mult)
                    rstd = ap.tile([128, 1], FP32, tag='rstd')
                    nc.scalar.activation(out=rstd, in_=mv[:, 1:2], func=AF.Sqrt,
                                         bias=eps1, scale=1.0)
                    nc.vector.reciprocal(out=rstd, in_=rstd)
                    nc.vector.tensor_copy(out=nmu_c[:, 64 + i:64 + i + 1], in_=rstd)
                    nmr = ap.tile([128, 1], FP32, tag='nmr')
                    nc.vector.tensor_scalar(out=nmr, in0=mv[:, 0:1], scalar1=rstd,
                                            scalar2=-1.0, op0=AluOpType.mult,
                                            op1=AluOpType.mult)
                    if has_g1b1:
                        tmp = ap.tile([128, E], FP32, tag='qtmp')
                        nc.vector.tensor_scalar(out=tmp, in0=xt, scalar1=mv[:, 0:1],
                                                scalar2=rstd, op0=AluOpType.subtract,
                                                op1=AluOpType.mult)
                        nc.vector.tensor_mul(tmp, tmp, g1b)
                        nc.vector.tensor_tensor(out=qn_sb[:, i, :], in0=tmp, in1=b1b,
                                                op=AluOpType.add)
                    else:
                        nc.scalar.activation(out=qn_sb[:, i, :], in_=xt,
                                             func=AF.Identity, scale=rstd, bias=nmr)

                # [-mu | rstd] cols -> token-major rows (via DRAM, tiny):
                # [128, 16] --transpose--> [16, 128] contiguous in DRAM, then
                # read back as [1, 1024] rows.
                nm128 = ap.tile([128, 128], BF, tag='nm128')
                nc.sync.dma_start(out=nm128, in_=nmu_c, transpose=True)
                nc.sync.dma_start(out=nm_dram[0:NT, :], in_=nm128[0:NT, :])
                nc.sync.dma_start(out=nm_dram[NT:2 * NT, :],
                                  in_=nm128[64:64 + NT, :])
                nc.sync.dma_start(out=nmu_row,
                                  in_=nm_dram[0:NT, :].rearrange('a b -> (a b)'))
                nc.sync.dma_start(out=rstd_row,
                                  in_=nm_dram[NT:2 * NT, :].rearrange('a b -> (a b)'))


            # ============ Phase B: v GEMM, then k GEMM + kv interleaved ====
            with (
                tc.tile_pool(name='psB', bufs=5, space='PSUM') as psb,
                tc.tile_pool(name='psC', bufs=1, space='PSUM') as psc,
            ):
                # kv accumulators: 3 banks (6+6+4 heads), padded to 2KB
                kvps = [psc.tile([128, 512], FP32, tag=f'kv{b}', name=f'kv{b}')
                        for b in range(3)]
                KVSLOT = [(h // 6, (h % 6) * 65) for h in range(H)]

                def v_tile(i, ch):
                    csl = slice(ch * 512, (ch + 1) * 512)
                    pv = psb.tile([128, 512], FP32, tag='psB', name=f'pv_{i}_{ch}')
                    for k in range(NK):
                        nc.tensor.matmul(pv, lhsT=xfm[:, k, i * 128:(i + 1) * 128],
                                         rhs=wv_t[:, k, csl],
                                         start=(k == 0), stop=(k == NK - 1))
                    if has_vb:
                        nc.vector.tensor_tensor(out=pv, in0=pv, in1=vbb[:, csl],
                                                op=AluOpType.add)
                    nc.scalar.activation(
                        out=v_aug[:, i, ch * 8:(ch + 1) * 8, 0:64],
                        in_=pv[:].rearrange('p (h d) -> p h d', d=64),
                        func=AF.Copy)

                def k_tile(i, ch):
                    csl = slice(ch * 512, (ch + 1) * 512)
                    pk = psb.tile([128, 512], FP32, tag='psB', name=f'pk_{i}_{ch}')
                    for k in range(NK):
                        nc.tensor.matmul(pk, lhsT=xfm[:, k, i * 128:(i + 1) * 128],
                                         rhs=wk_t[:, k, csl],
                                         start=(k == 0), stop=(k == NK - 1))
                    if has_kb:
                        nc.vector.tensor_tensor(out=pk, in0=pk, in1=kbb[:, csl],
                                                op=AluOpType.add)
                    pkv = pk[:].rearrange('p (h d) -> p h d', d=64)
                    # relu(k)*s on scalar (s, rstd > 0 commute with relu)
                    nc.scalar.activation(
                        out=ksc[:, i, ch * 8:(ch + 1) * 8, 0:64], in_=pkv,
                        func=AF.Relu, scale=scol[:, i:i + 1])
                    # relu(k)*c on vector (reads PSUM; keeps scalar queue free)
                    nc.vector.tensor_scalar(
                        out=ksc[:, i, ch * 8:(ch + 1) * 8, 64:128], in0=pkv,
                        scalar1=0.0, scalar2=ccol[:, i:i + 1],
                        op0=AluOpType.max, op1=AluOpType.mult)

                for i in range(NT):
                    v_tile(i, 0)
                    v_tile(i, 1)

                for p in range(4):
                    i0, i1 = 2 * p, 2 * p + 1
                    k_tile(i0, 0)
                    k_tile(i0, 1)
                    k_tile(i1, 0)
                    k_tile(i1, 1)
                    for i in (i0, i1):
                        for h in range(H):
                            b, off = KVSLOT[h]
                            nc.tensor.matmul(
                                kvps[b][:, off:off + 65],
                                lhsT=ksc[:, i, h, :], rhs=v_aug[:, i, h, :],
                                start=(i == 0), stop=(i == NT - 1),
                                skip_group_check=True)

                # kv psum -> bf16 sbuf -> DRAM -> AllReduce
                kvp = pp.tile([128, H * 65], BF, tag='kvp')
                for b in range(3):
                    nh = 6 if b < 2 else 4
                    nc.scalar.activation(out=kvp[:, b * 390:b * 390 + nh * 65],
                                         in_=kvps[b][:, 0:nh * 65], func=AF.Copy)
                nc.gpsimd.dma_start(out=kv_cc_in[:], in_=kvp)
                nc.gpsimd.collective_compute(
                    'AllReduce', AluOpType.add,
                    ins=[kv_cc_in.opt()], outs=[kv_cc_out.opt()],
                    replica_groups=RG)
                # kvb load on sync: the gpsimd queue must not block on the
                # collective (wq + attn-epilogue work are queued behind it)
                nc.sync.dma_start(out=kvb, in_=kv_cc_out[:])

            # wq on gpsimd (reuses wv slot; waits until B1 readers done)
            wq_t = wp.tile([128, NK, E], BF, tag='W', name='wq')
            wq_src = d_wq[:].rearrange('(k p) e -> p k e', p=128)
            for k in range(NK):
                nc.gpsimd.dma_start(out=wq_t[:, k, :], in_=wq_src[:, k, :])
            # wo on scalar (reuses wk slot; waits until k GEMM done)
            wo_t = wp.tile([128, NK, E], BF, tag='W', name='wo')
            wo_src = d_wo[:].rearrange('(k p) e -> p k e', p=128)
            for k in range(NK):
                nc.scalar.dma_start(out=wo_t[:, k, :], in_=wo_src[:, k, :])

            # rstd broadcast + folded sin/cos multipliers (needed by the q
            # epilogue; emitted here so the gpsimd queue is not blocked
            # waiting on LN1 stats during the v GEMM)
            nc.gpsimd.partition_broadcast(rstd_b, rstd_row)
            if has_qb:
                # generic path applies rstd explicitly; plain s/c tiles
                nc.vector.tensor_copy(out=srt, in_=sbt)
                nc.vector.tensor_copy(out=crt, in_=cbt)
            else:
                nc.vector.tensor_tensor(out=srt, in0=sbt, in1=rstd_b,
                                        op=AluOpType.mult)
                nc.vector.tensor_tensor(out=crt, in0=cbt, in1=rstd_b,
                                        op=AluOpType.mult)

            # ============ Phase Bq: q GEMM (W-stationary on x_fm) =========
            with (
                tc.tile_pool(name='psQ', bufs=5, space='PSUM') as psq,
                tc.tile_pool(name='qsb', bufs=4) as qsp,
            ):
                for j in range(NJ):
                    for ch in range(2):
                        csl = slice(ch * 512, (ch + 1) * 512)
                        pq = psq.tile([128, 512], FP32, tag='psQ',
                                      name=f'pq_{j}_{ch}')
                        for k in range(NK):
                            nc.tensor.matmul(pq,
                                             lhsT=wq_t[:, k, j * 128:(j + 1) * 128],
                                             rhs=xfm[:, k, csl],
                                             start=(k == 0), stop=False)
                        # rank-1 LN1-mean correction: pq += cq[j-chunk] x (-mu)
                        nc.tensor.matmul(pq, lhsT=cq_sb[0:1, j * 128:(j + 1) * 128],
                                         rhs=nmu_row[0:1, csl],
                                         start=False, stop=True)
                        qrel = qsp.tile([128, 512], BF, tag='qrel')
                        if has_qb:
                            nc.vector.tensor_tensor(out=pq, in0=pq,
                                                    in1=rstd_b[:, csl],
                                                    op=AluOpType.mult)
                            nc.scalar.activation(out=qrel, in_=pq, func=AF.Relu,
                                                 bias=qbc[:, j:j + 1])
                        else:
                            nc.scalar.activation(out=qrel, in_=pq, func=AF.Relu)
                        # gpsimd gets the partition-aligned writes, vector the
                        # partition-shifted ones (shift proven on DVE only)
                        nc.vector.tensor_tensor(
                            out=qq[0:64, 2 * j, csl], in0=qrel[0:64, :],
                            in1=srt[0:64, csl], op=AluOpType.mult)
                        nc.vector.tensor_tensor(
                            out=qq[64:128, 2 * j, csl], in0=qrel[0:64, :],
                            in1=crt[0:64, csl], op=AluOpType.mult)
                        nc.vector.tensor_tensor(
                            out=qq[0:64, 2 * j + 1, csl], in0=qrel[64:128, :],
                            in1=srt[64:128, csl], op=AluOpType.mult)
                        nc.vector.tensor_tensor(
                            out=qq[64:128, 2 * j + 1, csl], in0=qrel[64:128, :],
                            in1=crt[64:128, csl], op=AluOpType.mult)

            # ============ Phases E (attn+LN2), T (transpose), G (out) =====
            with (
                tc.tile_pool(name='ef', bufs=3) as efp,
                tc.tile_pool(name='psE', bufs=5, space='PSUM') as pse,
                tc.tile_pool(name='go', bufs=4) as gop,
                tc.tile_pool(name='psG', bufs=3, space='PSUM') as psg,
            ):
                def emit_attn_ln2(i):
                    rsl = slice(i * 128, (i + 1) * 128)
                    yt = efp.tile([128, E], BF, tag='yt')
                    dcol = efp.tile([128, H], FP32, tag='dcol')
                    z16 = efp.tile([128, H], FP32, tag='z16')
                    pas = []
                    for g in range(4):
                        pa = pse.tile([128, 512], FP32, tag='psE', name=f'pa_{i}_{g}')
                        pas.append(pa)
                        for hh in range(4):
                            h = 4 * g + hh
                            nc.tensor.matmul(pa[:, hh * 65:(hh + 1) * 65],
                                             lhsT=qq[:, h, rsl],
                                             rhs=kvb[:, h * 65:(h + 1) * 65],
                                             start=True, stop=True)
                        pav = pa[:, 0:260].rearrange('p (h c) -> p h c', c=65)
                        nc.scalar.activation(out=dcol[:, g * 4:(g + 1) * 4],
                                             in_=pav[:, :, 64], func=AF.Copy)
                    nc.vector.tensor_scalar(out=z16, in0=dcol, scalar1=EPS_ATTN,
                                            scalar2=None, op0=AluOpType.max)
                    nc.vector.reciprocal(out=z16, in_=z16)
                    ytv = yt[:].rearrange('p (h d) -> p h d', d=64)
                    for g in range(4):
                        pav = pas[g][:, 0:260].rearrange('p (h c) -> p h c', c=65)
                        zb = z16[:, g * 4:(g + 1) * 4].broadcast_to((128, 4, 64))
                        nc.vector.tensor_tensor(out=ytv[:, g * 4:(g + 1) * 4, :],
                                                in0=pav[:, :, 0:64], in1=zb,
                                                op=AluOpType.mult)
                    nc.vector.tensor_tensor(out=yt, in0=yt, in1=qn_sb[:, i, :],
                                            op=AluOpType.add)
                    # LN2
                    st2 = efp.tile([128, 2, 6], FP32, tag='st2')
                    yg = yt[:].rearrange('p (g d) -> p g d', g=2)
                    nc.vector.bn_stats(out=st2[:, 0, :], in_=yg[:, 0, :])
                    nc.vector.bn_stats(out=st2[:, 1, :], in_=yg[:, 1, :])
                    mv2 = efp.tile([128, 2], FP32, tag='mv2')
                    nc.vector.bn_aggr(out=mv2, in_=st2)
                    rstd2 = efp.tile([128, 1], FP32, tag='rstd2')
                    nc.scalar.activation(out=rstd2, in_=mv2[:, 1:2], func=AF.Sqrt,
                                         bias=eps1, scale=1.0)
                    nc.vector.reciprocal(out=rstd2, in_=rstd2)
                    nmr2 = efp.tile([128, 1], FP32, tag='nmr2')
                    nc.vector.tensor_scalar(out=nmr2, in0=mv2[:, 0:1], scalar1=rstd2,
                                            scalar2=-1.0, op0=AluOpType.mult,
                                            op1=AluOpType.mult)
                    xh = efp.tile([128, E], BF, tag='xh')
                    nc.scalar.activation(out=xh, in_=yt, func=AF.Identity,
                                         scale=rstd2, bias=nmr2)
                    nc.gpsimd.dma_start(out=xh_dram[rsl, :], in_=xh)

                def emit_xh_transpose(g):
                    # transpose tokens [256g, 256g+256) of xh, split over the
                    # two HWDGE queues
                    tsl = slice(g * 256, (g + 1) * 256)
                    for j in range(NJ):
                        nc.sync.dma_start(out=xhT[:, j, tsl],
                                          in_=xh_dram[tsl, j * 128:(j + 1) * 128],
                                          transpose=True)

                def emit_o(i):
                    pos = []
                    for ch in range(2):
                        po = psg.tile([128, 512], FP32, tag='psG',
                                      name=f'po_{i}_{ch}')
                        pos.append(po)
                    for k in range(NK):
                        for ch in range(2):
                            csl = slice(ch * 512, (ch + 1) * 512)
                            nc.tensor.matmul(pos[ch],
                                             lhsT=xhT[:, k, i * 128:(i + 1) * 128],
                                             rhs=wo_t[:, k, csl],
                                             start=(k == 0), stop=(k == NK - 1))
                    for ch in range(2):
                        csl = slice(ch * 512, (ch + 1) * 512)
                        ot = gop.tile([128, 512], FP32, tag='ot')
                        if has_b2o:
                            nc.vector.tensor_tensor(out=ot, in0=pos[ch],
                                                    in1=b2ob[:, csl],
                                                    op=AluOpType.add)
                        else:
                            nc.scalar.activation(out=ot, in_=pos[ch], func=AF.Copy)
                        nc.sync.dma_start(out=d_out[i * 128:(i + 1) * 128, csl],
                                          in_=ot)

                emit_attn_ln2(0)
                emit_attn_ln2(1)
                emit_xh_transpose(0)
                emit_attn_ln2(2)
                emit_o(0)
                emit_attn_ln2(3)
                emit_o(1)
                emit_xh_transpose(1)
                emit_attn_ln2(4)
                emit_o(2)
                emit_attn_ln2(5)
                emit_o(3)
                emit_xh_transpose(2)
                emit_attn_ln2(6)
                emit_o(4)
                emit_attn_ln2(7)
                emit_o(5)
                emit_xh_transpose(3)
                emit_o(6)
                emit_o(7)

    nc.compile()
    return nc


def _get_program(flags):
    if flags not in _BUILD_CACHE:
        _BUILD_CACHE[flags] = _build_program(flags)
    return _BUILD_CACHE[flags]


def _phm_weight(A, S):
    f = A.shape[0]
    din, dout = f * S.shape[1], f * S.shape[2]
    W = np.einsum('nij,nkl->ikjl', np.asarray(A, np.float32), np.asarray(S, np.float32))
    return np.ascontiguousarray(W.reshape(din, dout))


def kernel(**inputs):
    query = np.asarray(inputs['query'], np.float32)
    g1 = np.asarray(inputs['g1'], np.float32)
    b1 = np.asarray(inputs['b1'], np.float32)
    g2 = np.asarray(inputs['g2'], np.float32)
    b2 = np.asarray(inputs['b2'], np.float32)
    qb = np.asarray(inputs['qb'], np.float32)
    kb = np.asarray(inputs['kb'], np.float32)
    vb = np.asarray(inputs['vb'], np.float32)
    ob = np.asarray(inputs['ob'], np.float32)

    Wq = _phm_weight(inputs['qA'], inputs['qS'])
    Wk = _phm_weight(inputs['kA'], inputs['kS'])
    Wv = _phm_weight(inputs['vA'], inputs['vS'])
    Wo = _phm_weight(inputs['oA'], inputs['oS'])
    WoI = Wo + np.eye(E, dtype=np.float32)
    Wo2 = g2[:, None] * WoI
    B2O = b2 @ WoI + ob

    # fold LN1 affine into the q projection: qn*g1+b1 @ Wq
    Wq_eff = g1[:, None] * Wq
    qb_eff = qb + b1 @ Wq
    cq = Wq_eff.sum(axis=0)

    has_g1b1 = not (np.all(g1 == 1.0) and np.all(b1 == 0.0))
    has_qb = bool(np.any(qb_eff != 0.0))
    has_kb = bool(np.any(kb != 0.0))
    has_vb = bool(np.any(vb != 0.0))
    has_b2o = bool(np.any(B2O != 0.0))
    flags = (has_g1b1, has_qb, has_kb, has_vb, has_b2o)

    nc = _get_program(flags)

    s_full = np.sin((np.pi / 2) * np.arange(1, L + 1, dtype=np.float32) / L)
    c_full = np.cos((np.pi / 2) * np.arange(1, L + 1, dtype=np.float32) / L)

    wq_b = Wq_eff.astype(BF16)
    wk_b = Wk.astype(BF16)
    wv_b = Wv.astype(BF16)
    wo_b = Wo2.astype(BF16)

    in_maps = []
    for core in range(NCORES):
        b = core // 2
        l0 = (core % 2) * T
        x = np.ascontiguousarray(query[l0:l0 + T, b, :])
        s = s_full[l0:l0 + T]
        c = c_full[l0:l0 + T]
        im = {
            'x_tm': x,
            'x_fm': np.ascontiguousarray(x.T).astype(BF16),
            'wq': wq_b, 'wk': wk_b, 'wv': wv_b, 'wo2': wo_b,
            's_bcast': np.ascontiguousarray(np.broadcast_to(s, (128, T))).astype(BF16),
            'c_bcast': np.ascontiguousarray(np.broadcast_to(c, (128, T))).astype(BF16),
            's_cols': np.ascontiguousarray(s.reshape(NT, 128).T),
            'c_cols': np.ascontiguousarray(c.reshape(NT, 128).T),
            'cq_row': np.ascontiguousarray(cq.reshape(1, E)).astype(BF16),
        }
        if has_g1b1:
            im['g1_b'] = np.ascontiguousarray(np.broadcast_to(g1, (128, E)))
            im['b1_b'] = np.ascontiguousarray(np.broadcast_to(b1, (128, E)))
        if has_qb:
            im['qb_cols'] = np.ascontiguousarray(qb_eff.reshape(NJ, 128).T)
        if has_kb:
            im['kb_b'] = np.ascontiguousarray(np.broadcast_to(kb, (128, E)))
        if has_vb:
            im['vb_b'] = np.ascontiguousarray(np.broadcast_to(vb, (128, E)))
        if has_b2o:
            im['b2o_b'] = np.ascontiguousarray(np.broadcast_to(B2O, (128, E)))
        in_maps.append(im)

    trace = bool(os.environ.get('KERNEL_TRACE'))
    res = run_bass_kernel_spmd(nc, in_maps, list(range(NCORES)), trace=trace)
    kernel._last_exec_ns = res.exec_time_ns

    out = np.empty((L, N, E), np.float32)
    for core in range(NCORES):
        b = core // 2
        l0 = (core % 2) * T
        out[l0:l0 + T, b, :] = res.results[core]['out']
    return out


kernel._last_exec_ns = None


# revision 15
# speedup vs baseline: 1.3647x; 1.2799x over previous
"""Cosformer attention Bass kernel for 8 trn2 NeuronCores.

Sharding: core c handles batch c//2, sequence half c%2 (1024 tokens).
Per-head linear-attention state (kv, ksum) is AllReduce'd (bf16) between
the two cores sharing a batch.

v4 design:
- LN1 folded into the q GEMM: q runs W-stationary on the feature-major x
  plus a rank-1 cq (x) (-mu) correction matmul into the same PSUM group;
  rstd is folded into the sin/cos multiplier tiles (rstd, s, c > 0 commute
  with ReLU). No qn transposes.
- PE order: v GEMM, k GEMM with per-head kv matmuls interleaved per
  tile-pair (AllReduce triggers right after the k GEMM), q GEMM (covers
  the AllReduce), then per token tile: attn matmuls, PE-transpose of xh
  (identity matmul), output GEMM.
- Consecutive matmuls always alternate PSUM banks (pair loops) so the PE
  pipelines; kv/attn matmul orders are bank-interleaved.
- No gpsimd elementwise ops (Pool engine is ~10-25x slower than DVE).
  gpsimd does DMA issue, memset, partition_broadcast, collectives only.
- Scalar engine does all func(in*scale+bias) epilogues: v/k epilogues,
  qn and xh (LayerNorm apply via Identity with per-token scale/bias),
  relu(q), PSUM->SBUF copies. Vector does stats, sin/cos mults, z, y.
"""

import sys

for _p in ('/opt/trn_rl_repo',):
    if _p not in sys.path:
        sys.path.insert(0, _p)

import os

os.environ.setdefault('NEURON_RT_RESET_CORES', '1')

# The image's antenv may lack axon_hooks (needed for trace=True); register
# a stub module so `from antenv.axon_hooks import ...` works.
if 'antenv.axon_hooks' not in sys.modules:
    try:
        import antenv.axon_hooks  # noqa: F401
    except ImportError:
        import types as _types

        _mod = _types.ModuleType('antenv.axon_hooks')
        _mod._hook = None

        def _set_hook(h):
            _mod._hook = h

        def _get_hook():
            return _mod._hook

        _mod.set_axon_ntff_profile_hook = _set_hook
        _mod.get_axon_ntff_profile_hook = _get_hook
        sys.modules['antenv.axon_hooks'] = _mod

import numpy as np
import ml_dtypes

import concourse.bass as bass  # noqa: F401
import concourse.tile as tile
from concourse import bacc, mybir
from concourse.alu_op_type import AluOpType
from concourse.bass_utils import run_bass_kernel_spmd

BF16 = ml_dtypes.bfloat16
FP32 = mybir.dt.float32
BF = mybir.dt.bfloat16
AF = mybir.ActivationFunctionType

L, N, E, H, D = 2048, 4, 1024, 16, 64
T = 1024            # tokens per core
NT = T // 128       # 8 token tiles
NK = E // 128       # 8 contraction tiles
NJ = E // 128       # 8 output-feature tiles
NCORES = 8
EPS_LN = 1e-5
EPS_ATTN = 1e-6

# kv psum: 3 banks x (6|6|4) heads; emission order interleaves banks
KVSLOT = [(h // 6, (h % 6) * 65) for h in range(H)]
KV_ORDER = [0, 6, 12, 1, 7, 13, 2, 8, 14, 3, 9, 15, 4, 10, 5, 11]

_BUILD_CACHE = {}


def _build_program(flags):
    """Build the SPMD Bass program. flags: (has_g1b1, has_qb, has_kb, has_vb, has_b2o)."""
    has_g1b1, has_qb, has_kb, has_vb, has_b2o = flags

    nc = bacc.Bacc("TRN2", target_bir_lowering=False, debug=False,
                   num_devices=NCORES)

    # ---- DRAM I/O ----
    d_x_tm = nc.dram_tensor('x_tmb', [T, E], BF, kind='ExternalInput')
    d_x_fm = nc.dram_tensor('x_fm', [E, T], BF, kind='ExternalInput')
    d_wq = nc.dram_tensor('wq', [E, E], BF, kind='ExternalInput')
    d_wk = nc.dram_tensor('wk', [E, E], BF, kind='ExternalInput')
    d_wv = nc.dram_tensor('wv', [E, E], BF, kind='ExternalInput')
    d_wo = nc.dram_tensor('wo2', [E, E], BF, kind='ExternalInput')
    d_sb = nc.dram_tensor('s_bcast', [128, T], BF, kind='ExternalInput')
    d_cb = nc.dram_tensor('c_bcast', [128, T], BF, kind='ExternalInput')
    d_scol = nc.dram_tensor('s_cols', [128, NT], FP32, kind='ExternalInput')
    d_ccol = nc.dram_tensor('c_cols', [128, NT], FP32, kind='ExternalInput')
    d_cq = nc.dram_tensor('cq_row', [1, E], BF, kind='ExternalInput')
    d_ident = nc.dram_tensor('ident', [128, 128], BF, kind='ExternalInput')
    d_g1b = nc.dram_tensor('g1_b', [128, E], FP32, kind='ExternalInput') if has_g1b1 else None
    d_b1b = nc.dram_tensor('b1_b', [128, E], FP32, kind='ExternalInput') if has_g1b1 else None
    d_qbc = nc.dram_tensor('qb_cols', [128, NJ], FP32, kind='ExternalInput') if has_qb else None
    d_kbb = nc.dram_tensor('kb_b', [128, E], FP32, kind='ExternalInput') if has_kb else None
    d_vbb = nc.dram_tensor('vb_b', [128, E], FP32, kind='ExternalInput') if has_vb else None
    d_b2ob = nc.dram_tensor('b2o_b', [128, E], FP32, kind='ExternalInput') if has_b2o else None
    d_out = nc.dram_tensor('out', [T, E], FP32, kind='ExternalOutput')

    RG = [[0, 1], [2, 3], [4, 5], [6, 7]]

    with tile.TileContext(nc) as tc:
        with (
            tc.tile_pool(name='persist', bufs=1) as pp,
            tc.tile_pool(name='wpool', bufs=2) as wp,
            tc.tile_pool(name='dram', bufs=1, space='DRAM') as dp,
        ):
            # ---- priority DMAs: first GEMM inputs ----
            xfm = pp.tile([128, NK, T], BF, tag='xfm')
            xfm_src = d_x_fm[:].rearrange('(k p) t -> p k t', p=128)
            for k in range(NK):
                nc.sync.dma_start(out=xfm[:, k, :], in_=xfm_src[:, k, :])
            wv_t = wp.tile([128, NK, E], BF, tag='W', name='wv')
            wv_src = d_wv[:].rearrange('(k p) e -> p k e', p=128)
            for k in range(NK):
                nc.gpsimd.dma_start(out=wv_t[:, k, :], in_=wv_src[:, k, :])
            # x token-major (bf16) for LN1 stats + qn residual; scalar queue
            xt_sb = pp.tile([128, NT, E], BF, tag='xt')
            for i in range(NT):
                nc.scalar.dma_start(out=xt_sb[:, i, :],
                                    in_=d_x_tm[i * 128:(i + 1) * 128, :])

            # ---- constants (sync queue, after xfm) ----
            scol = pp.tile([128, NT], FP32, tag='scol')
            ccol = pp.tile([128, NT], FP32, tag='ccol')
            sbt = pp.tile([128, T], BF, tag='sbt')
            cbt = pp.tile([128, T], BF, tag='cbt')
            cq_sb = pp.tile([1, E], BF, tag='cq')
            ident = pp.tile([128, 128], BF, tag='ident')
            eps1 = pp.tile([128, 1], FP32, tag='eps1')
            nc.sync.dma_start(out=scol, in_=d_scol[:])
            nc.sync.dma_start(out=ccol, in_=d_ccol[:])
            nc.sync.dma_start(out=sbt, in_=d_sb[:])
            nc.sync.dma_start(out=cbt, in_=d_cb[:])
            nc.sync.dma_start(out=cq_sb, in_=d_cq[:])
            nc.sync.dma_start(out=ident, in_=d_ident[:])
            nc.vector.memset(eps1, EPS_LN)

            # wk on sync after consts (needed when the k GEMM starts)
            wk_t = wp.tile([128, NK, E], BF, tag='W', name='wk')
            wk_src = d_wk[:].rearrange('(k p) e -> p k e', p=128)
            for k in range(NK):
                nc.sync.dma_start(out=wk_t[:, k, :], in_=wk_src[:, k, :])

            g1b = b1b = qbc = kbb = vbb = b2ob = None
            if has_g1b1:
                g1b = pp.tile([128, E], FP32, tag='g1b')
                b1b = pp.tile([128, E], FP32, tag='b1b')
                nc.gpsimd.dma_start(out=g1b, in_=d_g1b[:])
                nc.gpsimd.dma_start(out=b1b, in_=d_b1b[:])
            if has_qb:
                qbc = pp.tile([128, NJ], FP32, tag='qbc')
                nc.gpsimd.dma_start(out=qbc, in_=d_qbc[:])
            if has_kb:
                kbb = pp.tile([128, E], FP32, tag='kbb')
                nc.gpsimd.dma_start(out=kbb, in_=d_kbb[:])
            if has_vb:
                vbb = pp.tile([128, E], FP32, tag='vbb')
                nc.gpsimd.dma_start(out=vbb, in_=d_vbb[:])
            if has_b2o:
                b2ob = pp.tile([128, E], FP32, tag='b2ob')
                nc.gpsimd.dma_start(out=b2ob, in_=d_b2ob[:])

            # ---- persistent activation tiles ----
            qn_sb = pp.tile([128, NT, E], BF, tag='qn')        # LN1(x) residual
            qq = pp.tile([128, H, T], BF, tag='qq')            # q_ per head, fm
            kvb = pp.tile([128, H * 65], BF, tag='kvb')        # reduced kv
            kvp = pp.tile([128, H * 65], BF, tag='kvp')        # local partial
            xhT = pp.tile([128, NK, T], BF, tag='xhT')         # xh feature-major
            nmu_c = pp.tile([128, 128], BF, tag='nmu_c')       # [-mu|pad|rstd|pad]
            nmu_row = pp.tile([1, T], BF, tag='nmu_row')       # -mu token-major
            rstd_row = pp.tile([1, T], BF, tag='rstd_row')
            rstd_b = pp.tile([128, T], BF, tag='rstd_b')       # rstd part-bcast
            srt = pp.tile([128, T], BF, tag='srt')             # s*rstd (or s)
            crt = pp.tile([128, T], BF, tag='crt')             # c*rstd (or c)
            mvs = pp.tile([128, NT, 2], FP32, tag='mvs')       # LN1 (mu, var)
            rstds = pp.tile([128, NT], FP32, tag='rstds')      # LN1 rstd cols
            nmrs = pp.tile([128, NT], FP32, tag='nmrs')        # -mu*rstd cols

            # ---- DRAM scratch ----
            nm_dram = dp.tile([2 * NT, 128], BF)     # transposed [-mu|rstd]
            kv_cc_in = dp.tile([128, H * 65], BF)
            kv_cc_out = dp.tile([128, H * 65], BF)

            nc.gpsimd.memset(nmu_c, 0.0)

            # ============ Phase A1: LN1 stats (vector only; overlaps B1) ===
            with tc.tile_pool(name='ln1', bufs=2) as ap:
                for i in range(NT):
                    st = ap.tile([128, 2, 6], FP32, tag='st')
                    xg = xt_sb[:, i, :].rearrange('p (g d) -> p g d', g=2)
                    nc.vector.bn_stats(out=st[:, 0, :], in_=xg[:, 0, :])
                    nc.vector.bn_stats(out=st[:, 1, :], in_=xg[:, 1, :])
                    nc.vector.bn_aggr(out=mvs[:, i, :], in_=st)
                    nc.vector.tensor_scalar(out=nmu_c[:, i:i + 1],
                                            in0=mvs[:, i, 0:1],
                                            scalar1=-1.0, scalar2=None,
                                            op0=AluOpType.mult)

                # ======== Phase B1: v GEMM (bank-alternating pairs) ========
                with (
                    tc.tile_pool(name='psB', bufs=5, space='PSUM') as psb,
                    tc.tile_pool(name='psC', bufs=1, space='PSUM') as psc,
                    tc.tile_pool(name='bphase', bufs=1) as bp,
                ):
                    kvps = [psc.tile([128, 512], FP32, tag=f'kv{b}', name=f'kv{b}')
                            for b in range(3)]
                    v_aug = bp.tile([128, NT, H, 65], BF, tag='vaug')
                    nc.gpsimd.memset(v_aug[:, :, :, 64:65], 1.0)

                    def gemm_pair(w_t, i, nametag):
                        p0 = psb.tile([128, 512], FP32, tag='psB',
                                      name=f'{nametag}_{i}_0')
                        p1 = psb.tile([128, 512], FP32, tag='psB',
                                      name=f'{nametag}_{i}_1')
                        for k in range(NK):
                            nc.tensor.matmul(p0,
                                             lhsT=xfm[:, k, i * 128:(i + 1) * 128],
                                             rhs=w_t[:, k, 0:512],
                                             start=(k == 0), stop=(k == NK - 1))
                            nc.tensor.matmul(p1,
                                             lhsT=xfm[:, k, i * 128:(i + 1) * 128],
                                             rhs=w_t[:, k, 512:1024],
                                             start=(k == 0), stop=(k == NK - 1))
                        return p0, p1

                    def v_epilogue(i, ch, pv):
                        if has_vb:
                            csl = slice(ch * 512, (ch + 1) * 512)
                            nc.vector.tensor_tensor(out=pv, in0=pv, in1=vbb[:, csl],
                                                    op=AluOpType.add)
                        nc.scalar.activation(
                            out=v_aug[:, i, ch * 8:(ch + 1) * 8, 0:64],
                            in_=pv[:].rearrange('p (h d) -> p h d', d=64),
                            func=AF.Copy)

                    for i in range(NT):
                        p0, p1 = gemm_pair(wv_t, i, 'pv')
                        v_epilogue(i, 0, p0)
                        v_epilogue(i, 1, p1)

                    # ==== Phase A2: rstd (scalar sqrt after v-epilogues) ===
                    for i in range(NT):
                        nc.scalar.activation(out=rstds[:, i:i + 1],
                                             in_=mvs[:, i, 1:2], func=AF.Sqrt,
                                             bias=eps1, scale=1.0)
                    for i in range(NT):
                        nc.vector.reciprocal(out=rstds[:, i:i + 1],
                                             in_=rstds[:, i:i + 1])
                        nc.vector.tensor_copy(out=nmu_c[:, 64 + i:64 + i + 1],
                                              in_=rstds[:, i:i + 1])
                        nc.vector.tensor_scalar(out=nmrs[:, i:i + 1],
                                                in0=mvs[:, i, 0:1],
                                                scalar1=rstds[:, i:i + 1],
                                                scalar2=-1.0, op0=AluOpType.mult,
                                                op1=AluOpType.mult)

                    # [-mu | rstd] cols -> token-major [1, T] rows (via DRAM)
                    nm128 = ap.tile([128, 128], BF, tag='nm128')
                    nc.sync.dma_start(out=nm128, in_=nmu_c, transpose=True)
                    nc.sync.dma_start(out=nm_dram[0:NT, :], in_=nm128[0:NT, :])
                    nc.sync.dma_start(out=nm_dram[NT:2 * NT, :],
                                      in_=nm128[64:64 + NT, :])
                    nc.sync.dma_start(out=nmu_row,
                                      in_=nm_dram[0:NT, :].rearrange('a b -> (a b)'))
                    nc.sync.dma_start(out=rstd_row,
                                      in_=nm_dram[NT:2 * NT, :].rearrange('a b -> (a b)'))
                    nc.gpsimd.partition_broadcast(rstd_b, rstd_row)

                    # ==== Phase B2: k GEMM + kv accumulation interleaved ===
                    ksc = bp.tile([128, NT, H, 128], BF, tag='ksc')

                    def k_epilogue(i, ch, pk):
                        if has_kb:
                            csl = slice(ch * 512, (ch + 1) * 512)
                            nc.vector.tensor_tensor(out=pk, in0=pk, in1=kbb[:, csl],
                                                    op=AluOpType.add)
                        pkv = pk[:].rearrange('p (h d) -> p h d', d=64)
                        # relu(k)*s on scalar (s, c > 0 commute with relu)
                        nc.scalar.activation(
                            out=ksc[:, i, ch * 8:(ch + 1) * 8, 0:64], in_=pkv,
                            func=AF.Relu, scale=scol[:, i:i + 1])
                        # relu(k)*c on vector
                        nc.vector.tensor_scalar(
                            out=ksc[:, i, ch * 8:(ch + 1) * 8, 64:128], in0=pkv,
                            scalar1=0.0, scalar2=ccol[:, i:i + 1],
                            op0=AluOpType.max, op1=AluOpType.mult)

                    for i in range(NT):
                        p0, p1 = gemm_pair(wk_t, i, 'pk')
                        k_epilogue(i, 0, p0)
                        k_epilogue(i, 1, p1)
                        for h in KV_ORDER:
                            b, off = KVSLOT[h]
                            nc.tensor.matmul(
                                kvps[b][:, off:off + 65],
                                lhsT=ksc[:, i, h, :], rhs=v_aug[:, i, h, :],
                                start=(i == 0), stop=(i == NT - 1),
                                skip_group_check=True)

                    # kv psum -> bf16 sbuf -> DRAM -> AllReduce
                    for b in range(3):
                        nh = 6 if b < 2 else 4
                        nc.scalar.activation(out=kvp[:, b * 390:b * 390 + nh * 65],
                                             in_=kvps[b][:, 0:nh * 65], func=AF.Copy)
                    nc.gpsimd.dma_start(out=kv_cc_in[:], in_=kvp)
                    nc.gpsimd.collective_compute(
                        'AllReduce', AluOpType.add,
                        ins=[kv_cc_in.opt()], outs=[kv_cc_out.opt()],
                        replica_groups=RG)
                    # kvb load on sync: gpsimd must not block on the collective
                    nc.sync.dma_start(out=kvb, in_=kv_cc_out[:])

            # qn residual (scalar Identity: x*rstd + (-mu*rstd)); after the
            # B phase so the scalar queue never blocks B1/B2 epilogues
            if has_g1b1:
                with tc.tile_pool(name='qnp', bufs=2) as qnp:
                    for i in range(NT):
                        tmp = qnp.tile([128, E], FP32, tag='qtmp')
                        nc.vector.tensor_scalar(out=tmp, in0=xt_sb[:, i, :],
                                                scalar1=mvs[:, i, 0:1],
                                                scalar2=rstds[:, i:i + 1],
                                                op0=AluOpType.subtract,
                                                op1=AluOpType.mult)
                        nc.vector.tensor_mul(tmp, tmp, g1b)
                        nc.vector.tensor_tensor(out=qn_sb[:, i, :], in0=tmp,
                                                in1=b1b, op=AluOpType.add)
            else:
                for i in range(NT):
                    nc.scalar.activation(out=qn_sb[:, i, :], in_=xt_sb[:, i, :],
                                         func=AF.Identity,
                                         scale=rstds[:, i:i + 1],
                                         bias=nmrs[:, i:i + 1])

            # wq on gpsimd (reuses wv slot), wo on scalar (reuses wk slot)
            wq_t = wp.tile([128, NK, E], BF, tag='W', name='wq')
            wq_src = d_wq[:].rearrange('(k p) e -> p k e', p=128)
            for k in range(NK):
                nc.gpsimd.dma_start(out=wq_t[:, k, :], in_=wq_src[:, k, :])
            wo_t = wp.tile([128, NK, E], BF, tag='W', name='wo')
            wo_src = d_wo[:].rearrange('(k p) e -> p k e', p=128)
            for k in range(NK):
                nc.scalar.dma_start(out=wo_t[:, k, :], in_=wo_src[:, k, :])

            # folded sin/cos multipliers
            if has_qb:
                nc.vector.tensor_copy(out=srt, in_=sbt)
                nc.vector.tensor_copy(out=crt, in_=cbt)
            else:
                nc.vector.tensor_tensor(out=srt, in0=sbt, in1=rstd_b,
                                        op=AluOpType.mult)
                nc.vector.tensor_tensor(out=crt, in0=cbt, in1=rstd_b,
                                        op=AluOpType.mult)

            # ============ Phase Bq: q GEMM (W-stationary on x_fm) =========
            with (
                tc.tile_pool(name='psQ', bufs=5, space='PSUM') as psq,
                tc.tile_pool(name='qsb', bufs=4) as qsp,
            ):
                for j in range(NJ):
                    pq0 = psq.tile([128, 512], FP32, tag='psQ', name=f'pq_{j}_0')
                    pq1 = psq.tile([128, 512], FP32, tag='psQ', name=f'pq_{j}_1')
                    for k in range(NK):
                        nc.tensor.matmul(pq0,
                                         lhsT=wq_t[:, k, j * 128:(j + 1) * 128],
                                         rhs=xfm[:, k, 0:512],
                                         start=(k == 0), stop=False)
                        nc.tensor.matmul(pq1,
                                         lhsT=wq_t[:, k, j * 128:(j + 1) * 128],
                                         rhs=xfm[:, k, 512:1024],
                                         start=(k == 0), stop=False)
                    # rank-1 LN1-mean correction: pq += cq[j-chunk] (x) (-mu)
                    nc.tensor.matmul(pq0, lhsT=cq_sb[0:1, j * 128:(j + 1) * 128],
                                     rhs=nmu_row[0:1, 0:512],
                                     start=False, stop=True)
                    nc.tensor.matmul(pq1, lhsT=cq_sb[0:1, j * 128:(j + 1) * 128],
                                     rhs=nmu_row[0:1, 512:1024],
                                     start=False, stop=True)
                    for ch, pq in ((0, pq0), (1, pq1)):
                        csl = slice(ch * 512, (ch + 1) * 512)
                        qrel = qsp.tile([128, 512], BF, tag='qrel')
                        if has_qb:
                            nc.vector.tensor_tensor(out=pq, in0=pq,
                                                    in1=rstd_b[:, csl],
                                                    op=AluOpType.mult)
                            nc.scalar.activation(out=qrel, in_=pq, func=AF.Relu,
                                                 bias=qbc[:, j:j + 1])
                        else:
                            nc.scalar.activation(out=qrel, in_=pq, func=AF.Relu)
                        nc.vector.tensor_tensor(
                            out=qq[0:64, 2 * j, csl], in0=qrel[0:64, :],
                            in1=srt[0:64, csl], op=AluOpType.mult)
                        nc.vector.tensor_tensor(
                            out=qq[64:128, 2 * j, csl], in0=qrel[0:64, :],
                            in1=crt[0:64, csl], op=AluOpType.mult)
                        nc.vector.tensor_tensor(
                            out=qq[0:64, 2 * j + 1, csl], in0=qrel[64:128, :],
                            in1=srt[64:128, csl], op=AluOpType.mult)
                        nc.vector.tensor_tensor(
                            out=qq[64:128, 2 * j + 1, csl], in0=qrel[64:128, :],
                            in1=crt[64:128, csl], op=AluOpType.mult)

            # ============ Phases E (attn+LN2) / T (PE transpose) / G (out) =
            with (
                tc.tile_pool(name='ef', bufs=3) as efp,
                tc.tile_pool(name='psE', bufs=4, space='PSUM') as pse,
                tc.tile_pool(name='go', bufs=4) as gop,
                tc.tile_pool(name='psG', bufs=2, space='PSUM') as psg,
                tc.tile_pool(name='psT', bufs=2, space='PSUM') as pst,
            ):
                xh_tiles = {}

                def emit_attn_ln2(i):
                    rsl = slice(i * 128, (i + 1) * 128)
                    yt = efp.tile([128, E], BF, tag='yt')
                    dcol = efp.tile([128, H], FP32, tag='dcol')
                    z16 = efp.tile([128, H], FP32, tag='z16')
                    pas = [pse.tile([128, 512], FP32, tag='psE', name=f'pa_{i}_{g}')
                           for g in range(4)]
                    # bank-interleaved emission: head hh of each group first
                    for hh in range(4):
                        for g in range(4):
                            h = 4 * g + hh
                            nc.tensor.matmul(pas[g][:, hh * 65:(hh + 1) * 65],
                                             lhsT=qq[:, h, rsl],
                                             rhs=kvb[:, h * 65:(h + 1) * 65],
                                             start=True, stop=True)
                    for g in range(4):
                        pav = pas[g][:, 0:260].rearrange('p (h c) -> p h c', c=65)
                        nc.scalar.activation(out=dcol[:, g * 4:(g + 1) * 4],
                                             in_=pav[:, :, 64], func=AF.Copy)
                    nc.vector.tensor_scalar(out=z16, in0=dcol, scalar1=EPS_ATTN,
                                            scalar2=None, op0=AluOpType.max)
                    nc.vector.reciprocal(out=z16, in_=z16)
                    ytv = yt[:].rearrange('p (h d) -> p h d', d=64)
                    for g in range(4):
                        pav = pas[g][:, 0:260].rearrange('p (h c) -> p h c', c=65)
                        zb = z16[:, g * 4:(g + 1) * 4].broadcast_to((128, 4, 64))
                        nc.vector.tensor_tensor(out=ytv[:, g * 4:(g + 1) * 4, :],
                                                in0=pav[:, :, 0:64], in1=zb,
                                                op=AluOpType.mult)
                    nc.vector.tensor_tensor(out=yt, in0=yt, in1=qn_sb[:, i, :],
                                            op=AluOpType.add)
                    # LN2
                    st2 = efp.tile([128, 2, 6], FP32, tag='st2')
                    yg = yt[:].rearrange('p (g d) -> p g d', g=2)
                    nc.vector.bn_stats(out=st2[:, 0, :], in_=yg[:, 0, :])
                    nc.vector.bn_stats(out=st2[:, 1, :], in_=yg[:, 1, :])
                    mv2 = efp.tile([128, 2], FP32, tag='mv2')
                    nc.vector.bn_aggr(out=mv2, in_=st2)
                    rstd2 = efp.tile([128, 1], FP32, tag='rstd2')
                    nc.scalar.activation(out=rstd2, in_=mv2[:, 1:2], func=AF.Sqrt,
                                         bias=eps1, scale=1.0)
                    nc.vector.reciprocal(out=rstd2, in_=rstd2)
                    nmr2 = efp.tile([128, 1], FP32, tag='nmr2')
                    nc.vector.tensor_scalar(out=nmr2, in0=mv2[:, 0:1],
                                            scalar1=rstd2, scalar2=-1.0,
                                            op0=AluOpType.mult, op1=AluOpType.mult)
                    xh = efp.tile([128, E], BF, tag='xh')
                    nc.scalar.activation(out=xh, in_=yt, func=AF.Identity,
                                         scale=rstd2, bias=nmr2)
                    xh_tiles[i] = xh

                def emit_T(i):
                    # PE transpose of xh tile i into xhT (feature-major)
                    xh = xh_tiles.pop(i)
                    for j in range(NJ):
                        pt = pst.tile([128, 128], BF, tag='psT',
                                      name=f'pt_{i}_{j}')
                        nc.tensor.transpose(pt, xh[:, j * 128:(j + 1) * 128],
                                            ident)
                        nc.scalar.activation(out=xhT[:, j, i * 128:(i + 1) * 128],
                                             in_=pt, func=AF.Copy)

                def emit_o(i):
                    po0 = psg.tile([128, 512], FP32, tag='psG', name=f'po_{i}_0')
                    po1 = psg.tile([128, 512], FP32, tag='psG', name=f'po_{i}_1')
                    for k in range(NK):
                        nc.tensor.matmul(po0,
                                         lhsT=xhT[:, k, i * 128:(i + 1) * 128],
                                         rhs=wo_t[:, k, 0:512],
                                         start=(k == 0), stop=(k == NK - 1))
                        nc.tensor.matmul(po1,
                                         lhsT=xhT[:, k, i * 128:(i + 1) * 128],
                                         rhs=wo_t[:, k, 512:1024],
                                         start=(k == 0), stop=(k == NK - 1))
                    for ch, po in ((0, po0), (1, po1)):
                        csl = slice(ch * 512, (ch + 1) * 512)
                        ot = gop.tile([128, 512], FP32, tag='ot')
                        if has_b2o:
                            nc.vector.tensor_tensor(out=ot, in0=po,
                                                    in1=b2ob[:, csl],
                                                    op=AluOpType.add)
                        else:
                            nc.scalar.activation(out=ot, in_=po, func=AF.Copy)
                        nc.sync.dma_start(out=d_out[i * 128:(i + 1) * 128, csl],
                                          in_=ot)

                emit_attn_ln2(0)
                emit_T(0)
                for i in range(1, NT):
                    emit_attn_ln2(i)
                    emit_o(i - 1)
                    emit_T(i)
                emit_o(NT - 1)

    nc.compile()
    return nc


def _get_program(flags):
    if flags not in _BUILD_CACHE:
        _BUILD_CACHE[flags] = _build_program(flags)
    return _BUILD_CACHE[flags]


def _phm_weight(A, S):
    f = A.shape[0]
    din, dout = f * S.shape[1], f * S.shape[2]
    W = np.einsum('nij,nkl->ikjl', np.asarray(A, np.float32), np.asarray(S, np.float32))
    return np.ascontiguousarray(W.reshape(din, dout))


_IDENT = np.eye(128, dtype=BF16)


def kernel(**inputs):
    query = np.asarray(inputs['query'], np.float32)
    g1 = np.asarray(inputs['g1'], np.float32)
    b1 = np.asarray(inputs['b1'], np.float32)
    g2 = np.asarray(inputs['g2'], np.float32)
    b2 = np.asarray(inputs['b2'], np.float32)
    qb = np.asarray(inputs['qb'], np.float32)
    kb = np.asarray(inputs['kb'], np.float32)
    vb = np.asarray(inputs['vb'], np.float32)
    ob = np.asarray(inputs['ob'], np.float32)

    Wq = _phm_weight(inputs['qA'], inputs['qS'])
    Wk = _phm_weight(inputs['kA'], inputs['kS'])
    Wv = _phm_weight(inputs['vA'], inputs['vS'])
    Wo = _phm_weight(inputs['oA'], inputs['oS'])
    WoI = Wo + np.eye(E, dtype=np.float32)
    Wo2 = g2[:, None] * WoI
    B2O = b2 @ WoI + ob

    # fold LN1 affine into the q projection: (qn*g1+b1) @ Wq
    Wq_eff = g1[:, None] * Wq
    qb_eff = qb + b1 @ Wq
    cq = Wq_eff.sum(axis=0)

    has_g1b1 = not (np.all(g1 == 1.0) and np.all(b1 == 0.0))
    has_qb = bool(np.any(qb_eff != 0.0))
    has_kb = bool(np.any(kb != 0.0))
    has_vb = bool(np.any(vb != 0.0))
    has_b2o = bool(np.any(B2O != 0.0))
    flags = (has_g1b1, has_qb, has_kb, has_vb, has_b2o)

    nc = _get_program(flags)

    s_full = np.sin((np.pi / 2) * np.arange(1, L + 1, dtype=np.float32) / L)
    c_full = np.cos((np.pi / 2) * np.arange(1, L + 1, dtype=np.float32) / L)

    wq_b = Wq_eff.astype(BF16)
    wk_b = Wk.astype(BF16)
    wv_b = Wv.astype(BF16)
    wo_b = Wo2.astype(BF16)

    in_maps = []
    for core in range(NCORES):
        b = core // 2
        l0 = (core % 2) * T
        x = np.ascontiguousarray(query[l0:l0 + T, b, :])
        s = s_full[l0:l0 + T]
        c = c_full[l0:l0 + T]
        im = {
            'x_tmb': x.astype(BF16),
            'x_fm': np.ascontiguousarray(x.T).astype(BF16),
            'wq': wq_b, 'wk': wk_b, 'wv': wv_b, 'wo2': wo_b,
            's_bcast': np.ascontiguousarray(np.broadcast_to(s, (128, T))).astype(BF16),
            'c_bcast': np.ascontiguousarray(np.broadcast_to(c, (128, T))).astype(BF16),
            's_cols': np.ascontiguousarray(s.reshape(NT, 128).T),
            'c_cols': np.ascontiguousarray(c.reshape(NT, 128).T),
            'cq_row': np.ascontiguousarray(cq.reshape(1, E)).astype(BF16),
            'ident': _IDENT,
        }
        if has_g1b1:
            im['g1_b'] = np.ascontiguousarray(np.broadcast_to(g1, (128, E)))
            im['b1_b'] = np.ascontiguousarray(np.broadcast_to(b1, (128, E)))
        if has_qb:
            im['qb_cols'] = np.ascontiguousarray(qb_eff.reshape(NJ, 128).T)
        if has_kb:
            im['kb_b'] = np.ascontiguousarray(np.broadcast_to(kb, (128, E)))
        if has_vb:
            im['vb_b'] = np.ascontiguousarray(np.broadcast_to(vb, (128, E)))
        if has_b2o:
            im['b2o_b'] = np.ascontiguousarray(np.broadcast_to(B2O, (128, E)))
        in_maps.append(im)

    trace = bool(os.environ.get('KERNEL_TRACE'))
    res = run_bass_kernel_spmd(nc, in_maps, list(range(NCORES)), trace=trace)
    kernel._last_exec_ns = res.exec_time_ns

    out = np.empty((L, N, E), np.float32)
    for core in range(NCORES):
        b = core // 2
        l0 = (core % 2) * T
        out[l0:l0 + T, b, :] = res.results[core]['out']
    return out


kernel._last_exec_ns = None


# revision 16
# speedup vs baseline: 1.4116x; 1.0344x over previous
"""Cosformer attention Bass kernel for 8 trn2 NeuronCores.

Sharding: core c handles batch c//2, sequence half c%2 (1024 tokens).
Per-head linear-attention state (kv, ksum) is AllReduce'd (bf16) between
the two cores sharing a batch.

v4 design:
- LN1 folded into the q GEMM: q runs W-stationary on the feature-major x
  plus a rank-1 cq (x) (-mu) correction matmul into the same PSUM group;
  rstd is folded into the sin/cos multiplier tiles (rstd, s, c > 0 commute
  with ReLU). No qn transposes.
- PE order: v GEMM, k GEMM with per-head kv matmuls interleaved per
  tile-pair (AllReduce triggers right after the k GEMM), q GEMM (covers
  the AllReduce), then per token tile: attn matmuls, PE-transpose of xh
  (identity matmul), output GEMM.
- Consecutive matmuls always alternate PSUM banks (pair loops) so the PE
  pipelines; kv/attn matmul orders are bank-interleaved.
- No gpsimd elementwise ops (Pool engine is ~10-25x slower than DVE).
  gpsimd does DMA issue, memset, partition_broadcast, collectives only.
- Scalar engine does all func(in*scale+bias) epilogues: v/k epilogues,
  qn and xh (LayerNorm apply via Identity with per-token scale/bias),
  relu(q), PSUM->SBUF copies. Vector does stats, sin/cos mults, z, y.
"""

import sys

for _p in ('/opt/trn_rl_repo',):
    if _p not in sys.path:
        sys.path.insert(0, _p)

import os

os.environ.setdefault('NEURON_RT_RESET_CORES', '1')

# The image's antenv may lack axon_hooks (needed for trace=True); register
# a stub module so `from antenv.axon_hooks import ...` works.
if 'antenv.axon_hooks' not in sys.modules:
    try:
        import antenv.axon_hooks  # noqa: F401
    except ImportError:
        import types as _types

        _mod = _types.ModuleType('antenv.axon_hooks')
        _mod._hook = None

        def _set_hook(h):
            _mod._hook = h

        def _get_hook():
            return _mod._hook

        _mod.set_axon_ntff_profile_hook = _set_hook
        _mod.get_axon_ntff_profile_hook = _get_hook
        sys.modules['antenv.axon_hooks'] = _mod

import numpy as np
import ml_dtypes

import concourse.bass as bass  # noqa: F401
import concourse.tile as tile
from concourse import bacc, mybir
from concourse.alu_op_type import AluOpType
from concourse.bass_utils import run_bass_kernel_spmd

BF16 = ml_dtypes.bfloat16
FP32 = mybir.dt.float32
BF = mybir.dt.bfloat16
AF = mybir.ActivationFunctionType

L, N, E, H, D = 2048, 4, 1024, 16, 64
T = 1024            # tokens per core
NT = T // 128       # 8 token tiles
NK = E // 128       # 8 contraction tiles
NJ = E // 128       # 8 output-feature tiles
NCORES = 8
EPS_LN = 1e-5
EPS_ATTN = 1e-6

# kv psum: 3 banks x (6|6|4) heads; emission order interleaves banks
KVSLOT = [(h // 6, (h % 6) * 65) for h in range(H)]
KV_ORDER = [0, 6, 12, 1, 7, 13, 2, 8, 14, 3, 9, 15, 4, 10, 5, 11]

_BUILD_CACHE = {}


def _build_program(flags):
    """Build the SPMD Bass program. flags: (has_g1b1, has_qb, has_kb, has_vb, has_b2o)."""
    has_g1b1, has_qb, has_kb, has_vb, has_b2o = flags

    nc = bacc.Bacc("TRN2", target_bir_lowering=False, debug=False,
                   num_devices=NCORES)

    # ---- DRAM I/O ----
    d_x_tm = nc.dram_tensor('x_tmb', [T, E], BF, kind='ExternalInput')
    d_x_fm = nc.dram_tensor('x_fm', [E, T], BF, kind='ExternalInput')
    d_wq = nc.dram_tensor('wq', [E, E], BF, kind='ExternalInput')
    d_wk = nc.dram_tensor('wk', [E, E], BF, kind='ExternalInput')
    d_wv = nc.dram_tensor('wv', [E, E], BF, kind='ExternalInput')
    d_wo = nc.dram_tensor('wo2', [E, E], BF, kind='ExternalInput')
    d_sb = nc.dram_tensor('s_bcast', [128, T], BF, kind='ExternalInput')
    d_cb = nc.dram_tensor('c_bcast', [128, T], BF, kind='ExternalInput')
    d_scol = nc.dram_tensor('s_cols', [128, NT], FP32, kind='ExternalInput')
    d_ccol = nc.dram_tensor('c_cols', [128, NT], FP32, kind='ExternalInput')
    d_cq = nc.dram_tensor('cq_row', [1, E], BF, kind='ExternalInput')
    d_ident = nc.dram_tensor('ident', [128, 128], BF, kind='ExternalInput')
    d_g1b = nc.dram_tensor('g1_b', [128, E], FP32, kind='ExternalInput') if has_g1b1 else None
    d_b1b = nc.dram_tensor('b1_b', [128, E], FP32, kind='ExternalInput') if has_g1b1 else None
    d_qbc = nc.dram_tensor('qb_cols', [128, NJ], FP32, kind='ExternalInput') if has_qb else None
    d_kbb = nc.dram_tensor('kb_b', [128, E], FP32, kind='ExternalInput') if has_kb else None
    d_vbb = nc.dram_tensor('vb_b', [128, E], FP32, kind='ExternalInput') if has_vb else None
    d_b2ob = nc.dram_tensor('b2o_b', [128, E], FP32, kind='ExternalInput') if has_b2o else None
    d_out = nc.dram_tensor('out', [T, E], FP32, kind='ExternalOutput')

    RG = [[0, 1], [2, 3], [4, 5], [6, 7]]

    with tile.TileContext(nc) as tc:
        with (
            tc.tile_pool(name='persist', bufs=1) as pp,
            tc.tile_pool(name='wpool', bufs=2) as wp,
            tc.tile_pool(name='dram', bufs=1, space='DRAM') as dp,
        ):
            # ---- priority DMAs: first GEMM inputs ----
            xfm = pp.tile([128, NK, T], BF, tag='xfm')
            xfm_src = d_x_fm[:].rearrange('(k p) t -> p k t', p=128)
            for k in range(NK):
                nc.sync.dma_start(out=xfm[:, k, :], in_=xfm_src[:, k, :])
            wv_t = wp.tile([128, NK, E], BF, tag='W', name='wv')
            wv_src = d_wv[:].rearrange('(k p) e -> p k e', p=128)
            for k in range(NK):
                nc.gpsimd.dma_start(out=wv_t[:, k, :], in_=wv_src[:, k, :])
            # x token-major (bf16) for LN1 stats + qn residual; scalar queue
            xt_sb = pp.tile([128, NT, E], BF, tag='xt')
            for i in range(NT):
                nc.scalar.dma_start(out=xt_sb[:, i, :],
                                    in_=d_x_tm[i * 128:(i + 1) * 128, :])

            # ---- constants (sync queue, after xfm) ----
            scol = pp.tile([128, NT], FP32, tag='scol')
            ccol = pp.tile([128, NT], FP32, tag='ccol')
            sbt = pp.tile([128, T], BF, tag='sbt')
            cbt = pp.tile([128, T], BF, tag='cbt')
            cq_sb = pp.tile([1, E], BF, tag='cq')
            ident = pp.tile([128, 128], BF, tag='ident')
            eps1 = pp.tile([128, 1], FP32, tag='eps1')
            nc.sync.dma_start(out=scol, in_=d_scol[:])
            nc.sync.dma_start(out=ccol, in_=d_ccol[:])
            nc.sync.dma_start(out=sbt, in_=d_sb[:])
            nc.sync.dma_start(out=cbt, in_=d_cb[:])
            nc.sync.dma_start(out=cq_sb, in_=d_cq[:])
            nc.sync.dma_start(out=ident, in_=d_ident[:])
            nc.vector.memset(eps1, EPS_LN)

            # wk on sync after consts (needed when the k GEMM starts)
            wk_t = wp.tile([128, NK, E], BF, tag='W', name='wk')
            wk_src = d_wk[:].rearrange('(k p) e -> p k e', p=128)
            for k in range(NK):
                nc.sync.dma_start(out=wk_t[:, k, :], in_=wk_src[:, k, :])

            g1b = b1b = qbc = kbb = vbb = b2ob = None
            if has_g1b1:
                g1b = pp.tile([128, E], FP32, tag='g1b')
                b1b = pp.tile([128, E], FP32, tag='b1b')
                nc.gpsimd.dma_start(out=g1b, in_=d_g1b[:])
                nc.gpsimd.dma_start(out=b1b, in_=d_b1b[:])
            if has_qb:
                qbc = pp.tile([128, NJ], FP32, tag='qbc')
                nc.gpsimd.dma_start(out=qbc, in_=d_qbc[:])
            if has_kb:
                kbb = pp.tile([128, E], FP32, tag='kbb')
                nc.gpsimd.dma_start(out=kbb, in_=d_kbb[:])
            if has_vb:
                vbb = pp.tile([128, E], FP32, tag='vbb')
                nc.gpsimd.dma_start(out=vbb, in_=d_vbb[:])
            if has_b2o:
                b2ob = pp.tile([128, E], FP32, tag='b2ob')
                nc.gpsimd.dma_start(out=b2ob, in_=d_b2ob[:])

            # ---- persistent activation tiles ----
            qn_sb = pp.tile([128, NT, E], BF, tag='qn')        # LN1(x) residual
            qq = pp.tile([128, H, T], BF, tag='qq')            # q_ per head, fm
            kvb = pp.tile([128, H * 65], BF, tag='kvb')        # reduced kv
            kvp = pp.tile([128, H * 65], BF, tag='kvp')        # local partial
            xhT = pp.tile([128, NK, T], BF, tag='xhT')         # xh feature-major
            nmu_c = pp.tile([128, 128], BF, tag='nmu_c')       # [-mu|pad|rstd|pad]
            nmu_row = pp.tile([1, T], BF, tag='nmu_row')       # -mu token-major
            rstd_row = pp.tile([1, T], BF, tag='rstd_row')
            rstd_b = pp.tile([128, T], BF, tag='rstd_b')       # rstd part-bcast
            srt = pp.tile([128, T], BF, tag='srt')             # s*rstd (or s)
            crt = pp.tile([128, T], BF, tag='crt')             # c*rstd (or c)
            mvs = pp.tile([128, NT, 2], FP32, tag='mvs')       # LN1 (mu, var)
            rstds = pp.tile([128, NT], FP32, tag='rstds')      # LN1 rstd cols
            nmrs = pp.tile([128, NT], FP32, tag='nmrs')        # -mu*rstd cols

            # ---- DRAM scratch ----
            nm_dram = dp.tile([2 * NT, 128], BF)     # transposed [-mu|rstd]
            kv_cc_in = dp.tile([128, H * 65], BF)
            kv_cc_out = dp.tile([128, H * 65], BF)

            nc.gpsimd.memset(nmu_c, 0.0)

            # ============ Phase A1: LN1 stats (vector only; overlaps B1) ===
            with tc.tile_pool(name='ln1', bufs=2) as ap:
                for i in range(NT):
                    st = ap.tile([128, 2, 6], FP32, tag='st')
                    xg = xt_sb[:, i, :].rearrange('p (g d) -> p g d', g=2)
                    nc.vector.bn_stats(out=st[:, 0, :], in_=xg[:, 0, :])
                    nc.vector.bn_stats(out=st[:, 1, :], in_=xg[:, 1, :])
                    nc.vector.bn_aggr(out=mvs[:, i, :], in_=st)
                    nc.vector.tensor_scalar(out=nmu_c[:, i:i + 1],
                                            in0=mvs[:, i, 0:1],
                                            scalar1=-1.0, scalar2=None,
                                            op0=AluOpType.mult)

                # ======== Phase B1: v GEMM (bank-alternating pairs) ========
                with (
                    tc.tile_pool(name='psB', bufs=5, space='PSUM') as psb,
                    tc.tile_pool(name='psC', bufs=1, space='PSUM') as psc,
                    tc.tile_pool(name='bphase', bufs=1) as bp,
                ):
                    kvps = [psc.tile([128, 512], FP32, tag=f'kv{b}', name=f'kv{b}')
                            for b in range(3)]
                    v_aug = bp.tile([128, NT, H, 65], BF, tag='vaug')
                    nc.gpsimd.memset(v_aug[:, :, :, 64:65], 1.0)

                    def gemm_quad(w_t, i0, nametag):
                        # 2 token tiles x 2 chunks, k-outer: consumes weight
                        # chunk k once per step (matches DMA arrival), and
                        # consecutive matmuls alternate PSUM banks
                        ps = [psb.tile([128, 512], FP32, tag='psB',
                                       name=f'{nametag}_{i0}_{x}')
                              for x in range(4)]
                        for k in range(NK):
                            for x in range(4):
                                i, ch = i0 + x // 2, x % 2
                                nc.tensor.matmul(
                                    ps[x],
                                    lhsT=xfm[:, k, i * 128:(i + 1) * 128],
                                    rhs=w_t[:, k, ch * 512:(ch + 1) * 512],
                                    start=(k == 0), stop=(k == NK - 1))
                        return ps

                    def v_epilogue(i, ch, pv):
                        if has_vb:
                            csl = slice(ch * 512, (ch + 1) * 512)
                            nc.vector.tensor_tensor(out=pv, in0=pv, in1=vbb[:, csl],
                                                    op=AluOpType.add)
                        nc.scalar.activation(
                            out=v_aug[:, i, ch * 8:(ch + 1) * 8, 0:64],
                            in_=pv[:].rearrange('p (h d) -> p h d', d=64),
                            func=AF.Copy)

                    for i0 in range(0, NT, 2):
                        ps = gemm_quad(wv_t, i0, 'pv')
                        for x in range(4):
                            v_epilogue(i0 + x // 2, x % 2, ps[x])

                    # ==== Phase A2: rstd (scalar sqrt after v-epilogues) ===
                    for i in range(NT):
                        nc.scalar.activation(out=rstds[:, i:i + 1],
                                             in_=mvs[:, i, 1:2], func=AF.Sqrt,
                                             bias=eps1, scale=1.0)
                    for i in range(NT):
                        nc.vector.reciprocal(out=rstds[:, i:i + 1],
                                             in_=rstds[:, i:i + 1])
                        nc.vector.tensor_copy(out=nmu_c[:, 64 + i:64 + i + 1],
                                              in_=rstds[:, i:i + 1])
                        nc.vector.tensor_scalar(out=nmrs[:, i:i + 1],
                                                in0=mvs[:, i, 0:1],
                                                scalar1=rstds[:, i:i + 1],
                                                scalar2=-1.0, op0=AluOpType.mult,
                                                op1=AluOpType.mult)

                    # [-mu | rstd] cols -> token-major [1, T] rows (via DRAM)
                    nm128 = ap.tile([128, 128], BF, tag='nm128')
                    nc.sync.dma_start(out=nm128, in_=nmu_c, transpose=True)
                    nc.sync.dma_start(out=nm_dram[0:NT, :], in_=nm128[0:NT, :])
                    nc.sync.dma_start(out=nm_dram[NT:2 * NT, :],
                                      in_=nm128[64:64 + NT, :])
                    nc.sync.dma_start(out=nmu_row,
                                      in_=nm_dram[0:NT, :].rearrange('a b -> (a b)'))
                    nc.sync.dma_start(out=rstd_row,
                                      in_=nm_dram[NT:2 * NT, :].rearrange('a b -> (a b)'))
                    nc.gpsimd.partition_broadcast(rstd_b, rstd_row)

                    # ==== Phase B2: k GEMM + kv accumulation interleaved ===
                    ksc = bp.tile([128, NT, H, 128], BF, tag='ksc')

                    def k_epilogue(i, ch, pk):
                        if has_kb:
                            csl = slice(ch * 512, (ch + 1) * 512)
                            nc.vector.tensor_tensor(out=pk, in0=pk, in1=kbb[:, csl],
                                                    op=AluOpType.add)
                        pkv = pk[:].rearrange('p (h d) -> p h d', d=64)
                        # relu(k)*s on scalar (s, c > 0 commute with relu)
                        nc.scalar.activation(
                            out=ksc[:, i, ch * 8:(ch + 1) * 8, 0:64], in_=pkv,
                            func=AF.Relu, scale=scol[:, i:i + 1])
                        # relu(k)*c on vector
                        nc.vector.tensor_scalar(
                            out=ksc[:, i, ch * 8:(ch + 1) * 8, 64:128], in0=pkv,
                            scalar1=0.0, scalar2=ccol[:, i:i + 1],
                            op0=AluOpType.max, op1=AluOpType.mult)

                    for i0 in range(0, NT, 2):
                        ps = gemm_quad(wk_t, i0, 'pk')
                        # the kv chain gates the AllReduce: high priority so
                        # the scheduler doesn't defer it behind the q GEMM
                        with tc.high_priority():
                            for x in range(4):
                                k_epilogue(i0 + x // 2, x % 2, ps[x])
                            for i in (i0, i0 + 1):
                                for h in KV_ORDER:
                                    b, off = KVSLOT[h]
                                    nc.tensor.matmul(
                                        kvps[b][:, off:off + 65],
                                        lhsT=ksc[:, i, h, :],
                                        rhs=v_aug[:, i, h, :],
                                        start=(i == 0), stop=(i == NT - 1),
                                        skip_group_check=True)

                    # kv psum -> bf16 sbuf -> DRAM -> AllReduce
                    tc_hp = tc.high_priority()
                    tc_hp.__enter__()
                    for b in range(3):
                        nh = 6 if b < 2 else 4
                        nc.scalar.activation(out=kvp[:, b * 390:b * 390 + nh * 65],
                                             in_=kvps[b][:, 0:nh * 65], func=AF.Copy)
                    nc.gpsimd.dma_start(out=kv_cc_in[:], in_=kvp)
                    nc.gpsimd.collective_compute(
                        'AllReduce', AluOpType.add,
                        ins=[kv_cc_in.opt()], outs=[kv_cc_out.opt()],
                        replica_groups=RG)
                    # kvb load on sync: gpsimd must not block on the collective
                    nc.sync.dma_start(out=kvb, in_=kv_cc_out[:])
                    tc_hp.__exit__(None, None, None)

            # qn residual (scalar Identity: x*rstd + (-mu*rstd)); after the
            # B phase so the scalar queue never blocks B1/B2 epilogues
            if has_g1b1:
                with tc.tile_pool(name='qnp', bufs=2) as qnp:
                    for i in range(NT):
                        tmp = qnp.tile([128, E], FP32, tag='qtmp')
                        nc.vector.tensor_scalar(out=tmp, in0=xt_sb[:, i, :],
                                                scalar1=mvs[:, i, 0:1],
                                                scalar2=rstds[:, i:i + 1],
                                                op0=AluOpType.subtract,
                                                op1=AluOpType.mult)
                        nc.vector.tensor_mul(tmp, tmp, g1b)
                        nc.vector.tensor_tensor(out=qn_sb[:, i, :], in0=tmp,
                                                in1=b1b, op=AluOpType.add)
            else:
                for i in range(NT):
                    nc.scalar.activation(out=qn_sb[:, i, :], in_=xt_sb[:, i, :],
                                         func=AF.Identity,
                                         scale=rstds[:, i:i + 1],
                                         bias=nmrs[:, i:i + 1])

            # wq on gpsimd (reuses wv slot), wo on scalar (reuses wk slot)
            wq_t = wp.tile([128, NK, E], BF, tag='W', name='wq')
            wq_src = d_wq[:].rearrange('(k p) e -> p k e', p=128)
            for k in range(NK):
                nc.gpsimd.dma_start(out=wq_t[:, k, :], in_=wq_src[:, k, :])
            wo_t = wp.tile([128, NK, E], BF, tag='W', name='wo')
            wo_src = d_wo[:].rearrange('(k p) e -> p k e', p=128)
            for k in range(NK):
                nc.scalar.dma_start(out=wo_t[:, k, :], in_=wo_src[:, k, :])

            # folded sin/cos multipliers
            if has_qb:
                nc.vector.tensor_copy(out=srt, in_=sbt)
                nc.vector.tensor_copy(out=crt, in_=cbt)
            else:
                nc.vector.tensor_tensor(out=srt, in0=sbt, in1=rstd_b,
                                        op=AluOpType.mult)
                nc.vector.tensor_tensor(out=crt, in0=cbt, in1=rstd_b,
                                        op=AluOpType.mult)

            # ============ Phase Bq: q GEMM (W-stationary on x_fm) =========
            with (
                tc.tile_pool(name='psQ', bufs=5, space='PSUM') as psq,
                tc.tile_pool(name='qsb', bufs=4) as qsp,
            ):
                for j in range(NJ):
                    pq0 = psq.tile([128, 512], FP32, tag='psQ', name=f'pq_{j}_0')
                    pq1 = psq.tile([128, 512], FP32, tag='psQ', name=f'pq_{j}_1')
                    for k in range(NK):
                        nc.tensor.matmul(pq0,
                                         lhsT=wq_t[:, k, j * 128:(j + 1) * 128],
                                         rhs=xfm[:, k, 0:512],
                                         start=(k == 0), stop=False)
                        nc.tensor.matmul(pq1,
                                         lhsT=wq_t[:, k, j * 128:(j + 1) * 128],
                                         rhs=xfm[:, k, 512:1024],
                                         start=(k == 0), stop=False)
                    # rank-1 LN1-mean correction: pq += cq[j-chunk] (x) (-mu)
                    nc.tensor.matmul(pq0, lhsT=cq_sb[0:1, j * 128:(j + 1) * 128],
                                     rhs=nmu_row[0:1, 0:512],
                                     start=False, stop=True)
                    nc.tensor.matmul(pq1, lhsT=cq_sb[0:1, j * 128:(j + 1) * 128],
                                     rhs=nmu_row[0:1, 512:1024],
                                     start=False, stop=True)
                    for ch, pq in ((0, pq0), (1, pq1)):
                        csl = slice(ch * 512, (ch + 1) * 512)
                        qrel = qsp.tile([128, 512], BF, tag='qrel')
                        if has_qb:
                            nc.vector.tensor_tensor(out=pq, in0=pq,
                                                    in1=rstd_b[:, csl],
                                                    op=AluOpType.mult)
                            nc.scalar.activation(out=qrel, in_=pq, func=AF.Relu,
                                                 bias=qbc[:, j:j + 1])
                        else:
                            nc.scalar.activation(out=qrel, in_=pq, func=AF.Relu)
                        nc.vector.tensor_tensor(
                            out=qq[0:64, 2 * j, csl], in0=qrel[0:64, :],
                            in1=srt[0:64, csl], op=AluOpType.mult)
                        nc.vector.tensor_tensor(
                            out=qq[64:128, 2 * j, csl], in0=qrel[0:64, :],
                            in1=crt[0:64, csl], op=AluOpType.mult)
                        nc.vector.tensor_tensor(
                            out=qq[0:64, 2 * j + 1, csl], in0=qrel[64:128, :],
                            in1=srt[64:128, csl], op=AluOpType.mult)
                        nc.vector.tensor_tensor(
                            out=qq[64:128, 2 * j + 1, csl], in0=qrel[64:128, :],
                            in1=crt[64:128, csl], op=AluOpType.mult)

            # ============ Phases E (attn+LN2) / T (PE transpose) / G (out) =
            with (
                tc.tile_pool(name='ef', bufs=3) as efp,
                tc.tile_pool(name='psE', bufs=4, space='PSUM') as pse,
                tc.tile_pool(name='go', bufs=4) as gop,
                tc.tile_pool(name='psG', bufs=2, space='PSUM') as psg,
                tc.tile_pool(name='psT', bufs=2, space='PSUM') as pst,
            ):
                xh_tiles = {}

                def emit_attn_ln2(i):
                    rsl = slice(i * 128, (i + 1) * 128)
                    yt = efp.tile([128, E], BF, tag='yt')
                    dcol = efp.tile([128, H], FP32, tag='dcol')
                    z16 = efp.tile([128, H], FP32, tag='z16')
                    pas = [pse.tile([128, 512], FP32, tag='psE', name=f'pa_{i}_{g}')
                           for g in range(4)]
                    # bank-interleaved emission: head hh of each group first
                    for hh in range(4):
                        for g in range(4):
                            h = 4 * g + hh
                            nc.tensor.matmul(pas[g][:, hh * 65:(hh + 1) * 65],
                                             lhsT=qq[:, h, rsl],
                                             rhs=kvb[:, h * 65:(h + 1) * 65],
                                             start=True, stop=True)
                    for g in range(4):
                        pav = pas[g][:, 0:260].rearrange('p (h c) -> p h c', c=65)
                        nc.scalar.activation(out=dcol[:, g * 4:(g + 1) * 4],
                                             in_=pav[:, :, 64], func=AF.Copy)
                    nc.vector.tensor_scalar(out=z16, in0=dcol, scalar1=EPS_ATTN,
                                            scalar2=None, op0=AluOpType.max)
                    nc.vector.reciprocal(out=z16, in_=z16)
                    ytv = yt[:].rearrange('p (h d) -> p h d', d=64)
                    for g in range(4):
                        pav = pas[g][:, 0:260].rearrange('p (h c) -> p h c', c=65)
                        zb = z16[:, g * 4:(g + 1) * 4].broadcast_to((128, 4, 64))
                        nc.vector.tensor_tensor(out=ytv[:, g * 4:(g + 1) * 4, :],
                                                in0=pav[:, :, 0:64], in1=zb,
                                                op=AluOpType.mult)
                    nc.vector.tensor_tensor(out=yt, in0=yt, in1=qn_sb[:, i, :],
                                            op=AluOpType.add)
                    # LN2
                    st2 = efp.tile([128, 2, 6], FP32, tag='st2')
                    yg = yt[:].rearrange('p (g d) -> p g d', g=2)
                    nc.vector.bn_stats(out=st2[:, 0, :], in_=yg[:, 0, :])
                    nc.vector.bn_stats(out=st2[:, 1, :], in_=yg[:, 1, :])
                    mv2 = efp.tile([128, 2], FP32, tag='mv2')
                    nc.vector.bn_aggr(out=mv2, in_=st2)
                    rstd2 = efp.tile([128, 1], FP32, tag='rstd2')
                    nc.scalar.activation(out=rstd2, in_=mv2[:, 1:2], func=AF.Sqrt,
                                         bias=eps1, scale=1.0)
                    nc.vector.reciprocal(out=rstd2, in_=rstd2)
                    nmr2 = efp.tile([128, 1], FP32, tag='nmr2')
                    nc.vector.tensor_scalar(out=nmr2, in0=mv2[:, 0:1],
                                            scalar1=rstd2, scalar2=-1.0,
                                            op0=AluOpType.mult, op1=AluOpType.mult)
                    xh = efp.tile([128, E], BF, tag='xh')
                    nc.scalar.activation(out=xh, in_=yt, func=AF.Identity,
                                         scale=rstd2, bias=nmr2)
                    xh_tiles[i] = xh

                def emit_T(i):
                    # PE transpose of xh tile i into xhT (feature-major)
                    xh = xh_tiles.pop(i)
                    for j in range(NJ):
                        pt = pst.tile([128, 128], BF, tag='psT',
                                      name=f'pt_{i}_{j}')
                        nc.tensor.transpose(pt, xh[:, j * 128:(j + 1) * 128],
                                            ident)
                        nc.vector.tensor_copy(
                            out=xhT[:, j, i * 128:(i + 1) * 128], in_=pt)

                def emit_o(i):
                    po0 = psg.tile([128, 512], FP32, tag='psG', name=f'po_{i}_0')
                    po1 = psg.tile([128, 512], FP32, tag='psG', name=f'po_{i}_1')
                    for k in range(NK):
                        nc.tensor.matmul(po0,
                                         lhsT=xhT[:, k, i * 128:(i + 1) * 128],
                                         rhs=wo_t[:, k, 0:512],
                                         start=(k == 0), stop=(k == NK - 1))
                        nc.tensor.matmul(po1,
                                         lhsT=xhT[:, k, i * 128:(i + 1) * 128],
                                         rhs=wo_t[:, k, 512:1024],
                                         start=(k == 0), stop=(k == NK - 1))
                    for ch, po in ((0, po0), (1, po1)):
                        csl = slice(ch * 512, (ch + 1) * 512)
                        ot = gop.tile([128, 512], FP32, tag='ot')
                        if has_b2o:
                            nc.vector.tensor_tensor(out=ot, in0=po,
                                                    in1=b2ob[:, csl],
                                                    op=AluOpType.add)
                        else:
                            nc.scalar.activation(out=ot, in_=po, func=AF.Copy)
                        oeng = nc.sync if ch == 0 else nc.gpsimd
                        oeng.dma_start(out=d_out[i * 128:(i + 1) * 128, csl],
                                       in_=ot)

                emit_attn_ln2(0)
                emit_T(0)
                for i in range(1, NT):
                    emit_attn_ln2(i)
                    emit_o(i - 1)
                    emit_T(i)
                emit_o(NT - 1)

    nc.compile()
    return nc


def _get_program(flags):
    if flags not in _BUILD_CACHE:
        _BUILD_CACHE[flags] = _build_program(flags)
    return _BUILD_CACHE[flags]


def _phm_weight(A, S):
    f = A.shape[0]
    din, dout = f * S.shape[1], f * S.shape[2]
    W = np.einsum('nij,nkl->ikjl', np.asarray(A, np.float32), np.asarray(S, np.float32))
    return np.ascontiguousarray(W.reshape(din, dout))


_IDENT = np.eye(128, dtype=BF16)


def kernel(**inputs):
    query = np.asarray(inputs['query'], np.float32)
    g1 = np.asarray(inputs['g1'], np.float32)
    b1 = np.asarray(inputs['b1'], np.float32)
    g2 = np.asarray(inputs['g2'], np.float32)
    b2 = np.asarray(inputs['b2'], np.float32)
    qb = np.asarray(inputs['qb'], np.float32)
    kb = np.asarray(inputs['kb'], np.float32)
    vb = np.asarray(inputs['vb'], np.float32)
    ob = np.asarray(inputs['ob'], np.float32)

    Wq = _phm_weight(inputs['qA'], inputs['qS'])
    Wk = _phm_weight(inputs['kA'], inputs['kS'])
    Wv = _phm_weight(inputs['vA'], inputs['vS'])
    Wo = _phm_weight(inputs['oA'], inputs['oS'])
    WoI = Wo + np.eye(E, dtype=np.float32)
    Wo2 = g2[:, None] * WoI
    B2O = b2 @ WoI + ob

    # fold LN1 affine into the q projection: (qn*g1+b1) @ Wq
    Wq_eff = g1[:, None] * Wq
    qb_eff = qb + b1 @ Wq
    cq = Wq_eff.sum(axis=0)

    has_g1b1 = not (np.all(g1 == 1.0) and np.all(b1 == 0.0))
    has_qb = bool(np.any(qb_eff != 0.0))
    has_kb = bool(np.any(kb != 0.0))
    has_vb = bool(np.any(vb != 0.0))
    has_b2o = bool(np.any(B2O != 0.0))
    flags = (has_g1b1, has_qb, has_kb, has_vb, has_b2o)

    nc = _get_program(flags)

    s_full = np.sin((np.pi / 2) * np.arange(1, L + 1, dtype=np.float32) / L)
    c_full = np.cos((np.pi / 2) * np.arange(1, L + 1, dtype=np.float32) / L)

    wq_b = Wq_eff.astype(BF16)
    wk_b = Wk.astype(BF16)
    wv_b = Wv.astype(BF16)
    wo_b = Wo2.astype(BF16)

    in_maps = []
    for core in range(NCORES):
        b = core // 2
        l0 = (core % 2) * T
        x = np.ascontiguousarray(query[l0:l0 + T, b, :])
        s = s_full[l0:l0 + T]
        c = c_full[l0:l0 + T]
        im = {
            'x_tmb': x.astype(BF16),
            'x_fm': np.ascontiguousarray(x.T).astype(BF16),
            'wq': wq_b, 'wk': wk_b, 'wv': wv_b, 'wo2': wo_b,
            's_bcast': np.ascontiguousarray(np.broadcast_to(s, (128, T))).astype(BF16),
            'c_bcast': np.ascontiguousarray(np.broadcast_to(c, (128, T))).astype(BF16),
            's_cols': np.ascontiguousarray(s.reshape(NT, 128).T),
            'c_cols': np.ascontiguousarray(c.reshape(NT, 128).T),
            'cq_row': np.ascontiguousarray(cq.reshape(1, E)).astype(BF16),
            'ident': _IDENT,
        }
        if has_g1b1:
            im['g1_b'] = np.ascontiguousarray(np.broadcast_to(g1, (128, E)))
            im['b1_b'] = np.ascontiguousarray(np.broadcast_to(b1, (128, E)))
        if has_qb:
            im['qb_cols'] = np.ascontiguousarray(qb_eff.reshape(NJ, 128).T)
        if has_kb:
            im['kb_b'] = np.ascontiguousarray(np.broadcast_to(kb, (128, E)))
        if has_vb:
            im['vb_b'] = np.ascontiguousarray(np.broadcast_to(vb, (128, E)))
        if has_b2o:
            im['b2o_b'] = np.ascontiguousarray(np.broadcast_to(B2O, (128, E)))
        in_maps.append(im)

    trace = bool(os.environ.get('KERNEL_TRACE'))
    res = run_bass_kernel_spmd(nc, in_maps, list(range(NCORES)), trace=trace)
    kernel._last_exec_ns = res.exec_time_ns

    out = np.empty((L, N, E), np.float32)
    for core in range(NCORES):
        b = core // 2
        l0 = (core % 2) * T
        out[l0:l0 + T, b, :] = res.results[core]['out']
    return out


kernel._last_exec_ns = None
